# revision 14
# baseline (speedup 1.0000x reference)
"""Trainium2 Bass kernel for nn_AttnBlock (GroupNorm -> 1x1 q/k/v -> attention -> proj -> residual).

Input x: [4, 512, 64, 64] f32. Sharding: 8 cores = 4 batches x 2 query-halves.
Each core gets its batch's full x (columns permuted so its query half is first),
computes GroupNorm, attention over all 4096 keys for its 2048 queries, and
returns [512, 2048].

Weight-folding (exact algebra, done on host):
  scores^T[j,i] = (Wk hn_j + bk) . (Wq hn_i + bq)
               = hn_j . u_i + (const in j)   with u = (Wk^T Wq) hn + Wk^T bq
  The j-constant shifts every score of a query equally -> softmax-invariant,
  so the k projection, its converts, and bk disappear; scores contract the
  persistent fp8 hn directly against u.
  out = x + Wp(V attn + bv) + bp = x + Sum_j (Wp Wv hn_j) attn[j,:] + bpe
  with bpe = Wp bv + bp, so attn@v with vT' = (Wp Wv) hn directly produces
  projected output channels and the separate projection stage disappears.

Engine budget (cost model: ACT 0.833 ns/row +init, DVE 1.042 (0.5x all-SBUF
TensorScalar), Pool 0.833 SBUF-only, PE DR-fp8 107 ns per 512-row matmul):
  PE   ~72 us: stats mm + v'/u projections + scores + den + attn@v
  ACT  ~67 us: softmax exp only, two j-tiles per instruction ([P,2x512] PSUM)
  DVE  ~64 us: bn_stats, v'/u converts, o2*rb, reciprocal
  Pool ~21 us: hn fp8 conversion, fin = (o2*rb + bpe) + x, all SBUF-side
The denominator matmul uses an all-ones lhsT with M=128 so its PSUM result is
partition-broadcast; one DVE reciprocal yields the [128,512] normalizer,
applied to o2 before the (already folded) projection. Residual comes from the
resident bf16 x. GPSIMD cannot touch PSUM, so all PSUM readers are ACT/DVE.

Layouts (per core):
  x_sb  [128, 4, 4096]    bf16  (stats + hn source + residual)
  hn_f8 [128, 2, 2, 4096] fp8   c-pair-packed; lhsT for scores^T AND rhs for v'/u
  u_f8  [128, 2, 2, 2048] fp8   c-pair-packed rhs for scores^T
  vT_f8 [128, 16, 2, 512] fp8   j-pair-packed lhsT for attn@v (holds Wp Wv hn)
PSUM (8 banks): sc 2x[P,2,512] scores pairs + o2 2x[P,512] attn@v chains +
pp 2x[P,512] (stats mm, v'/u projections, den).
"""

import numpy as np
import ml_dtypes

import concourse.bass as bass
import concourse.mybir as mybir
import concourse.tile as tile
from concourse.vector_clock import ScopedClock
from concourse.bass_utils import run_bass_kernel_spmd

F32 = mybir.dt.float32
BF16 = mybir.dt.bfloat16
FP8 = mybir.dt.float8e4
AF = mybir.ActivationFunctionType
ALU = mybir.AluOpType

P = 128
C = 512          # channels
N = 4096         # spatial positions (64*64)
NQ = 2048        # queries per core (half)
CT = C // P      # 4 channel tiles
JC = N // 512    # 8 key chunks of 512
JT = N // P      # 32 key tiles of 128
TT = JT // 2     # 16 j-tile pairs
ICH = NQ // 512  # 4 query chunks of 512
NUM_GROUPS = 16
GSIZE = C // NUM_GROUPS            # 32 channels per group
EPS = 1e-6
SCALE = float(C) ** -0.5


class PatchedTileContext(tile.TileContext):
    """walrus in this container accepts only ONE sync-wait per instruction;
    split extra waits onto same-engine NoOps placed just before the
    instruction (same queue => waits still execute before it)."""

    def _lower_ordered_insts(self, ordered):
        for bb_name, insts in list(ordered.items()):
            new_list = []
            for inst in insts:
                si = inst.sync_info
                if si is not None and si.on_wait and len(si.on_wait) > 1:
                    waits = list(si.on_wait)
                    for w in waits[:-1]:
                        nop = mybir.InstNoOp(
                            name=self.nc.get_next_instruction_name(),
                            engine=inst.engine,
                            sync_info=mybir.SyncInfo(on_wait=[w], on_update=[]),
                            bass_nofuse=True,
                        )
                        new_list.append(nop)
                    si.on_wait = [waits[-1]]
                new_list.append(inst)
            ordered[bb_name] = new_list
        super()._lower_ordered_insts(ordered)

    def _drain_and_barrier(self, tick_clock, wait_clock):
        drain_inst = self.nc.sync.drain()
        wait_clock.add_sem_waits(
            drain_inst.ins, ScopedClock({None: tick_clock.global_clock})
        )
        si = drain_inst.ins.sync_info
        if si is not None and si.on_wait and len(si.on_wait) > 1:
            waits = list(si.on_wait)
            si.on_wait = [waits[0]]
            for w in waits[1:]:
                d2 = self.nc.sync.drain()
                d2.ins.sync_info = mybir.SyncInfo(on_wait=[w], on_update=[])
        self.nc.all_engine_barrier()
        assert self.sems is not None
        popped = self.nc._tile_sem_poison_stack.pop()
        assert popped is self._sem_poison
        self.nc.clear_and_free_semaphores(list(self.sems.allocated().values()))
        self.nc.all_engine_barrier()


def build_nc(reps=1):
    nc = bass.Bass(name=f"attnblk_r{reps}")

    xbf_d = nc.dram_tensor("xbf", [C, N], BF16, kind="ExternalInput")
    wkqtf8_d = nc.dram_tensor("wkqtf8", [P, 4 * 512], FP8, kind="ExternalInput")
    wpvtf8_d = nc.dram_tensor("wpvtf8", [P, 4 * 512], FP8, kind="ExternalInput")
    gamma_d = nc.dram_tensor("gamma", [C], F32, kind="ExternalInput")
    beta_d = nc.dram_tensor("beta", [C], F32, kind="ExternalInput")
    bu_d = nc.dram_tensor("bu", [C], F32, kind="ExternalInput")
    bpe_d = nc.dram_tensor("bpe", [C], F32, kind="ExternalInput")
    g4_d = nc.dram_tensor("g4", [P, 4], F32, kind="ExternalInput")
    g4t_d = nc.dram_tensor("g4t", [4, P], F32, kind="ExternalInput")
    out_d = nc.dram_tensor("out", [C, NQ], F32, kind="ExternalOutput")

    with PatchedTileContext(nc) as tc:
        with (
            tc.tile_pool(name="const", bufs=1) as const,
            tc.tile_pool(name="persist", bufs=1) as persist,
            tc.tile_pool(name="small", bufs=4) as small,
            tc.tile_pool(name="atp", bufs=36) as atp,
            tc.tile_pool(name="finp", bufs=6) as finp,
            tc.tile_pool(name="ps", bufs=1, space="PSUM") as ps,
        ):
            # ---------------- x load first: 3 DMA queues ----------------
            x_sb = persist.tile([P, CT, N], BF16)
            x_queues = [nc.sync, nc.scalar, nc.gpsimd]
            for piece in range(8):
                ct, xh = piece // 2, piece % 2
                x_queues[piece % 3].dma_start(
                    x_sb[:, ct, xh * 2048:(xh + 1) * 2048],
                    xbf_d[ct * P:(ct + 1) * P, xh * 2048:(xh + 1) * 2048],
                )

            # ---------------- constants (scalar queue, after x) --------------
            wkqt_f8 = const.tile([P, 2, 2, C], FP8)
            nc.gpsimd.dma_start(wkqt_f8[:], wkqtf8_d[:, :].rearrange("p (kp s co) -> p kp s co", kp=2, s=2))
            wpvt_f8 = const.tile([P, 2, 2, C], FP8)
            nc.gpsimd.dma_start(wpvt_f8[:], wpvtf8_d[:, :].rearrange("p (kp s co) -> p kp s co", kp=2, s=2))

            gam = const.tile([P, CT], F32)
            nc.sync.dma_start(gam[:], gamma_d[:].rearrange("(t p) -> p t", p=P))
            bet = const.tile([P, CT], F32)
            nc.sync.dma_start(bet[:], beta_d[:].rearrange("(t p) -> p t", p=P))
            bu4 = const.tile([P, CT], F32)
            nc.sync.dma_start(bu4[:], bu_d[:].rearrange("(t p) -> p t", p=P))
            bpe4 = const.tile([P, CT], F32)
            nc.sync.dma_start(bpe4[:], bpe_d[:].rearrange("(t p) -> p t", p=P))
            g4_sb = const.tile([P, 4], F32)
            nc.sync.dma_start(g4_sb[:], g4_d[:, :])
            g4t_sb = const.tile([4, P], F32)
            nc.sync.dma_start(g4t_sb[:], g4t_d[:, :])

            eps_sb = const.tile([P, 1], F32)
            nc.vector.memset(eps_sb[:], EPS)
            bias_m1 = const.tile([P, 1], F32)
            nc.vector.memset(bias_m1[:], -1.0)
            ones_f8 = const.tile([P, 2, P], FP8)
            nc.vector.memset(ones_f8[:], 1.0)

            # ---------------- persistent tensors ----------------
            hn_f8 = persist.tile([P, CT // 2, 2, N], FP8)
            vT_f8 = persist.tile([P, TT, 2, 512], FP8)
            u_f8 = persist.tile([P, CT // 2, 2, NQ], FP8)

            for _rep in range(reps):
                # ---------------- phase 0: groupnorm stats ------------------
                # Per-ct pipelines: each channel tile's (scale, bias) is ready
                # as soon as its own stats land (4 local groups per ct, so the
                # group reduce/broadcast matmuls are ct-independent).
                # ct0-2 on DVE (bn_stats), ct3 on ACT (wide sum/sumsq accum).
                scale_c = [persist.tile([P, 1], F32, name=f"scale_c{ct}") for ct in range(CT)]
                bias_c = [persist.tile([P, 1], F32, name=f"bias_c{ct}") for ct in range(CT)]
                for ct in range(CT):
                    red = small.tile([P, 2], F32, tag="red", name=f"red_{ct}")
                    if ct == 3:
                        reds = small.tile([P, 4], F32, tag="reds", bufs=1)
                        sumsc = small.tile([P, 2048], F32, tag="sumsc", bufs=1)
                        for xh in range(2):
                            nc.scalar.activation(
                                sumsc[:], x_sb[:, ct, xh * 2048:(xh + 1) * 2048],
                                AF.Copy, accum_out=reds[:, xh:xh + 1],
                            )
                            nc.scalar.activation(
                                sumsc[:], x_sb[:, ct, xh * 2048:(xh + 1) * 2048],
                                AF.Square, accum_out=reds[:, 2 + xh:3 + xh],
                            )
                        nc.vector.tensor_tensor(red[:], reds[:, 0:4:2], reds[:, 1:4:2], ALU.add)
                        nc.vector.tensor_scalar_mul(red[:], red[:], 1.0 / N)
                    else:
                        bnst = small.tile([P, JC, 6], F32, tag="bnst", name=f"bnst_{ct}")
                        for jc in range(JC):
                            nc.vector.bn_stats(bnst[:, jc, :], x_sb[:, ct, jc * 512:(jc + 1) * 512])
                        mv = small.tile([P, 2], F32, tag="mv", name=f"mv_{ct}")
                        nc.vector.bn_aggr(mv[:], bnst[:])
                        msq = small.tile([P, 1], F32, tag="msq", name=f"msq_{ct}")
                        nc.scalar.activation(msq[:], mv[:, 0:1], AF.Square)
                        nc.scalar.copy(red[:, 0:1], mv[:, 0:1])
                        nc.vector.tensor_tensor(red[:, 1:2], mv[:, 1:2], msq[:], ALU.add)
                    gps = ps.tile([4, 2], F32, tag="pp", bufs=2, name=f"gps_{ct}")
                    nc.tensor.matmul(
                        gps[:], lhsT=g4_sb[:], rhs=red[:],
                        start=True, stop=True,
                    )
                    # mr = (mu | rstd) [4, 2] in SBUF for the broadcast matmul
                    mr = small.tile([4, 2], F32, tag="mr", name=f"mr_{ct}")
                    nc.scalar.copy(mr[:, 0:1], gps[:, 0:1])
                    msq2 = small.tile([4, 1], F32, tag="msq2", name=f"msq2_{ct}")
                    nc.scalar.activation(msq2[:], gps[:, 0:1], AF.Square)
                    var1 = small.tile([4, 1], F32, tag="var1", name=f"var1_{ct}")
                    nc.vector.tensor_tensor(var1[:], gps[:, 1:2], msq2[:], ALU.subtract)
                    std1 = small.tile([4, 1], F32, tag="std1", name=f"std1_{ct}")
                    nc.scalar.activation(std1[:], var1[:], AF.Sqrt, bias=eps_sb[0:4, :])
                    nc.vector.reciprocal(mr[:, 1:2], std1[:])
                    mrp = ps.tile([P, 2], F32, tag="pp", bufs=2, name=f"mrp_{ct}")
                    nc.tensor.matmul(
                        mrp[:], lhsT=g4t_sb[:], rhs=mr[:],
                        start=True, stop=True,
                    )
                    nc.vector.tensor_tensor(scale_c[ct][:], gam[:, ct:ct + 1], mrp[:, 1:2], ALU.mult)
                    tbc = small.tile([P, 1], F32, tag="tbc", name=f"tbc_{ct}")
                    nc.vector.tensor_tensor(tbc[:], mrp[:, 0:1], scale_c[ct][:], ALU.mult)
                    nc.vector.tensor_tensor(bias_c[ct][:], bet[:, ct:ct + 1], tbc[:], ALU.subtract)

                # ---------------- fused phase 1 + attention ----------------
                # Emission schedule: hn conversion (Pool) inline per jc;
                # vT' chains for jc<=5 inline, jc 6-7 deferred; den/attn@v/
                # normalize of chunk i drained as closures between the slices
                # of chunk i+1 so PE work spreads instead of bursting. u for
                # chunk i+1 is a drained closure too (pp ring is free of den
                # during slices since den accumulates post-loop).

                def hn_conv(jc):
                    for kc in range(CT):
                        nc.gpsimd.tensor_scalar(
                            hn_f8[:, kc // 2, kc % 2, jc * 512:(jc + 1) * 512],
                            x_sb[:, kc, jc * 512:(jc + 1) * 512],
                            scale_c[kc][:], bias_c[kc][:],
                            ALU.mult, ALU.add,
                        )

                def v_chains(jc, conv_eng):
                    for jl in range(4):
                        jt = jc * 4 + jl
                        pv = ps.tile([P, 512], F32, tag="pp", bufs=2, name=f"pv_{jc}_{jl}")
                        for kp in range(2):
                            nc.tensor.matmul(
                                pv[:], lhsT=hn_f8[:, kp, :, jt * P:(jt + 1) * P],
                                rhs=wpvt_f8[:, kp],
                                perf_mode=mybir.MatmulPerfMode.DoubleRow,
                                start=(kp == 0), stop=(kp == 1),
                            )
                        if conv_eng is nc.scalar:
                            nc.scalar.activation(vT_f8[:, jt // 2, jt % 2, :], pv[:], AF.Identity)
                        else:
                            nc.vector.tensor_copy(vT_f8[:, jt // 2, jt % 2, :], pv[:])

                def u_proj(ich):
                    # u = (Wk^T Wq) hn + bu for this query chunk
                    for co in range(CT):
                        pq = ps.tile([P, 512], F32, tag="pp", bufs=2, name=f"pq_{ich}_{co}")
                        for kp in range(2):
                            nc.tensor.matmul(
                                pq[:], lhsT=wkqt_f8[:, kp, :, co * P:(co + 1) * P],
                                rhs=hn_f8[:, kp, :, ich * 512:(ich + 1) * 512],
                                perf_mode=mybir.MatmulPerfMode.DoubleRow,
                                start=(kp == 0), stop=(kp == 1),
                            )
                        nc.vector.tensor_scalar(
                            u_f8[:, co // 2, co % 2, ich * 512:(ich + 1) * 512], pq[:],
                            bu4[:, co:co + 1], None, ALU.add,
                        )

                at2s = {ich: [] for ich in range(ICH)}
                rbs = {}

                def att_slice(ich, t):
                    # scores pair -> one exp (den accumulated post-loop)
                    sc = ps.tile([P, 2, 512], F32, tag="sc", bufs=2, name=f"sc_{ich}_{t}")
                    for s in range(2):
                        jt = 2 * t + s
                        for kp in range(2):
                            nc.tensor.matmul(
                                sc[:, s, :], lhsT=hn_f8[:, kp, :, jt * P:(jt + 1) * P],
                                rhs=u_f8[:, kp, :, ich * 512:(ich + 1) * 512],
                                perf_mode=mybir.MatmulPerfMode.DoubleRow,
                                start=(kp == 0), stop=(kp == 1),
                            )
                    at2 = atp.tile([P, 2, 512], FP8, tag="at", name=f"at2_{ich}_{t}")
                    at2s[ich].append(at2)
                    nc.scalar.activation(at2[:], sc[:], AF.Exp, scale=SCALE, bias=bias_m1[:])

                def den_rec(ich, psum_tag="pp"):
                    den = ps.tile([P, 512], F32, tag=psum_tag, bufs=2, name=f"den_{ich}")
                    for t in range(TT):
                        nc.tensor.matmul(
                            den[:], lhsT=ones_f8[:], rhs=at2s[ich][t][:],
                            perf_mode=mybir.MatmulPerfMode.DoubleRow,
                            start=(t == 0), stop=(t == TT - 1),
                        )
                    rb = finp.tile([P, 512], F32, tag="rb", name=f"rb_{ich}")
                    nc.vector.reciprocal(rb[:], den[:])
                    rbs[ich] = rb

                def av_chain(ich, ct, psum_tag="o2"):
                    o2t = ps.tile([P, 512], F32, tag=psum_tag, bufs=2, name=f"o2_{ich}_{ct}")
                    for t in range(TT):
                        nc.tensor.matmul(
                            o2t[:], lhsT=vT_f8[:, t, :, ct * P:(ct + 1) * P],
                            rhs=at2s[ich][t][:],
                            perf_mode=mybir.MatmulPerfMode.DoubleRow,
                            start=(t == 0), stop=(t == TT - 1),
                        )
                    t1 = finp.tile([P, 512], F32, tag="t1", name=f"t1_{ich}_{ct}")
                    nc.vector.tensor_tensor(t1[:], o2t[:], rbs[ich][:], ALU.mult)
                    fin = finp.tile([P, 512], F32, tag="fin", name=f"fin_{ich}_{ct}")
                    nc.vector.scalar_tensor_tensor(
                        fin[:], t1[:], bpe4[:, ct:ct + 1],
                        x_sb[:, ct, ich * 512:(ich + 1) * 512],
                        ALU.add, ALU.add,
                    )
                    out_q = nc.sync if ct % 2 == 0 else nc.scalar
                    out_q.dma_start(
                        out_d[ct * P:(ct + 1) * P, ich * 512:(ich + 1) * 512], fin[:]
                    )

                # --- chunk 0 window: phase 1 interleaved with its slices ---
                hn_conv(0)
                u_proj(0)
                v_chains(0, nc.vector)
                for jc in range(1, JC):
                    hn_conv(jc)
                    if jc <= 5:
                        v_chains(jc, nc.scalar if jc % 2 == 1 else nc.vector)
                    for t in (2 * (jc - 1), 2 * (jc - 1) + 1):
                        att_slice(0, t)
                for t in (2 * (JC - 1), 2 * (JC - 1) + 1):
                    att_slice(0, t)

                # --- chunks 1..3: slices with drained finish work ---
                pend = [lambda: v_chains(6, nc.vector),
                        lambda: v_chains(7, nc.scalar),
                        lambda: den_rec(0),
                        lambda: av_chain(0, 0), lambda: av_chain(0, 1),
                        lambda: u_proj(2),
                        lambda: av_chain(0, 2), lambda: av_chain(0, 3)]
                u_proj(1)
                last = ICH - 1
                tmaj = {}

                def tmaj_start(ct, upto):
                    o2t = ps.tile([P, 512], F32, tag="o2", bufs=2, name=f"o2_{last}_{ct}")
                    tmaj[ct] = o2t
                    for t in range(upto):
                        nc.tensor.matmul(
                            o2t[:], lhsT=vT_f8[:, t, :, ct * P:(ct + 1) * P],
                            rhs=at2s[last][t][:],
                            perf_mode=mybir.MatmulPerfMode.DoubleRow,
                            start=(t == 0), stop=False,
                        )

                for ich in range(1, ICH):
                    den_l = None
                    for t in range(TT):
                        att_slice(ich, t)
                        if ich == last:
                            if t == 4:
                                den_l = ps.tile([P, 512], F32, tag="pp", bufs=2,
                                                name=f"den_{last}")
                                for tp in range(5):
                                    nc.tensor.matmul(
                                        den_l[:], lhsT=ones_f8[:], rhs=at2s[last][tp][:],
                                        perf_mode=mybir.MatmulPerfMode.DoubleRow,
                                        start=(tp == 0), stop=False,
                                    )
                            elif t > 4:
                                nc.tensor.matmul(
                                    den_l[:], lhsT=ones_f8[:], rhs=at2s[last][t][:],
                                    perf_mode=mybir.MatmulPerfMode.DoubleRow,
                                    start=False, stop=(t == TT - 1),
                                )
                            if t == 12:
                                tmaj_start(0, 13)
                                tmaj_start(1, 13)
                            elif t > 12:
                                for ct in (0, 1):
                                    nc.tensor.matmul(
                                        tmaj[ct][:], lhsT=vT_f8[:, t, :, ct * P:(ct + 1) * P],
                                        rhs=at2s[last][t][:],
                                        perf_mode=mybir.MatmulPerfMode.DoubleRow,
                                        start=False, stop=(t == TT - 1),
                                    )
                        if t % 2 == 1 and pend:
                            pend.pop(0)()
                    while pend:
                        pend.pop(0)()
                    if ich < ICH - 1:
                        pend = [lambda i=ich: den_rec(i),
                                lambda i=ich: av_chain(i, 0),
                                lambda i=ich: av_chain(i, 1)]
                        if ich + 2 < ICH:
                            pend.append(lambda i=ich: u_proj(i + 2))
                        pend += [lambda i=ich: av_chain(i, 2),
                                 lambda i=ich: av_chain(i, 3)]
                # tail: reciprocal, finish t-major ct0/1, then ct2/3 on sc ring
                rb = finp.tile([P, 512], F32, tag="rb", name=f"rb_{last}")
                nc.vector.reciprocal(rb[:], den_l[:])
                rbs[last] = rb
                for ct in (0, 1):
                    t1 = finp.tile([P, 512], F32, tag="t1", name=f"t1_{last}_{ct}")
                    nc.vector.tensor_tensor(t1[:], tmaj[ct][:], rb[:], ALU.mult)
                    fin = finp.tile([P, 512], F32, tag="fin", name=f"fin_{last}_{ct}")
                    nc.vector.scalar_tensor_tensor(
                        fin[:], t1[:], bpe4[:, ct:ct + 1],
                        x_sb[:, ct, last * 512:(last + 1) * 512],
                        ALU.add, ALU.add,
                    )
                    out_q = nc.sync if ct % 2 == 0 else nc.scalar
                    out_q.dma_start(
                        out_d[ct * P:(ct + 1) * P, last * 512:(last + 1) * 512], fin[:]
                    )
                av_chain(last, 2, psum_tag="sc")
                av_chain(last, 3, psum_tag="sc")
    return nc


_NC = None


def _get_nc():
    global _NC
    if _NC is None:
        _NC = build_nc()
    return _NC


def _make_in_maps(x, gamma, beta, wq, bq, wk, bk, wv, bv, wp, bp):
    x = np.ascontiguousarray(np.asarray(x, dtype=np.float32)).reshape(4, C, N)
    bf = ml_dtypes.bfloat16

    def pack8(w):
        return np.ascontiguousarray(
            np.asarray(w, np.float32).T.reshape(2, 2, P, 512).transpose(2, 0, 1, 3)
            .reshape(P, 4 * 512).astype(mybir.dt.np(FP8))
        )

    g4i = np.zeros((P, 4), np.float32)
    for p in range(P):
        g4i[p, p // GSIZE] = 1.0
    g4 = g4i / GSIZE          # group-mean matmul (pre-scaled)
    g4t = np.ascontiguousarray(g4i.T)  # broadcast indicator (0/1)

    wq = np.asarray(wq, np.float32)
    wk = np.asarray(wk, np.float32)
    wv = np.asarray(wv, np.float32)
    wp = np.asarray(wp, np.float32)
    # scores fold: u = (Wk^T Wq) hn + Wk^T bq  (bk shift is softmax-invariant)
    wkq = wk.T @ wq
    bu = wk.T @ np.asarray(bq, np.float32)
    # output fold: attn@v with (Wp Wv) hn; bpe = Wp bv + bp
    wpv = wp @ wv
    bpe = np.asarray(bp, np.float32) + wp @ np.asarray(bv, np.float32)

    common = {
        "wkqtf8": pack8(wkq), "wpvtf8": pack8(wpv),
        "gamma": np.asarray(gamma, np.float32), "beta": np.asarray(beta, np.float32),
        "bu": np.ascontiguousarray(bu), "bpe": np.ascontiguousarray(bpe),
        "g4": g4, "g4t": g4t,
    }
    in_maps = []
    for core in range(8):
        bidx, half = core // 2, core % 2
        xb = x[bidx]
        if half == 0:
            xp = xb
        else:
            xp = np.concatenate([xb[:, NQ:], xb[:, :NQ]], axis=1)
        xp = np.ascontiguousarray(xp)
        in_maps.append({"xbf": xp.astype(bf), **common})
    return in_maps


def run(inputs, trace=False):
    nc = _get_nc()
    in_maps = _make_in_maps(**inputs)
    res = run_bass_kernel_spmd(nc, in_maps, list(range(8)), trace=trace)
    out = np.empty((4, C, N), np.float32)
    for core in range(8):
        bidx, half = core // 2, core % 2
        o = res.results[core]["out"]
        if half == 0:
            out[bidx, :, :NQ] = o
        else:
            out[bidx, :, NQ:] = o
    return out.reshape(4, C, 64, 64), res


def kernel(**inputs):
    out, _ = run(inputs, trace=False)
    return out


# revision 15
# speedup vs baseline: 1.0444x; 1.0444x over previous
"""Trainium2 Bass kernel for nn_AttnBlock (GroupNorm -> 1x1 q/k/v -> attention -> proj -> residual).

Input x: [4, 512, 64, 64] f32. Sharding: 8 cores = 4 batches x 2 query-halves.
Each core gets its batch's full x (columns permuted so its query half is first),
computes GroupNorm, attention over all 4096 keys for its 2048 queries, and
returns [512, 2048].

Weight-folding (exact algebra, done on host):
  scores^T[j,i] = (Wk hn_j + bk) . (Wq hn_i + bq)
               = hn_j . u_i + (const in j)   with u = (Wk^T Wq) hn + Wk^T bq
  The j-constant shifts every score of a query equally -> softmax-invariant,
  so the k projection, its converts, and bk disappear; scores contract the
  persistent fp8 hn directly against u.
  out = x + Wp(V attn + bv) + bp = x + Sum_j (Wp Wv hn_j) attn[j,:] + bpe
  with bpe = Wp bv + bp, so attn@v with vT' = (Wp Wv) hn directly produces
  projected output channels and the separate projection stage disappears.

Engine budget (cost model: ACT 0.833 ns/row +init, DVE 1.042 (0.5x all-SBUF
TensorScalar), Pool 0.833 SBUF-only, PE DR-fp8 107 ns per 512-row matmul):
  PE   ~72 us: stats mm + v'/u projections + scores + den + attn@v
  ACT  ~67 us: softmax exp only, two j-tiles per instruction ([P,2x512] PSUM)
  DVE  ~64 us: bn_stats, v'/u converts, o2*rb, reciprocal
  Pool ~21 us: hn fp8 conversion, fin = (o2*rb + bpe) + x, all SBUF-side
The denominator matmul uses an all-ones lhsT with M=128 so its PSUM result is
partition-broadcast; one DVE reciprocal yields the [128,512] normalizer,
applied to o2 before the (already folded) projection. Residual comes from the
resident bf16 x. GPSIMD cannot touch PSUM, so all PSUM readers are ACT/DVE.

Layouts (per core):
  x_sb  [128, 4, 4096]    bf16  (stats + hn source + residual)
  hn_f8 [128, 2, 2, 4096] fp8   c-pair-packed; lhsT for scores^T AND rhs for v'/u
  u_f8  [128, 2, 2, 2048] fp8   c-pair-packed rhs for scores^T
  vT_f8 [128, 16, 2, 512] fp8   j-pair-packed lhsT for attn@v (holds Wp Wv hn)
PSUM (8 banks): sc 2x[P,2,512] scores pairs + o2 2x[P,512] attn@v chains +
pp 2x[P,512] (stats mm, v'/u projections, den).
"""

import numpy as np
import ml_dtypes

import concourse.bass as bass
import concourse.mybir as mybir
import concourse.tile as tile
from concourse.vector_clock import ScopedClock
from concourse.bass_utils import run_bass_kernel_spmd

F32 = mybir.dt.float32
BF16 = mybir.dt.bfloat16
FP8 = mybir.dt.float8e4
AF = mybir.ActivationFunctionType
ALU = mybir.AluOpType

P = 128
C = 512          # channels
N = 4096         # spatial positions (64*64)
NQ = 2048        # queries per core (half)
CT = C // P      # 4 channel tiles
JC = N // 512    # 8 key chunks of 512
JT = N // P      # 32 key tiles of 128
TT = JT // 2     # 16 j-tile pairs
ICH = NQ // 512  # 4 query chunks of 512
NUM_GROUPS = 16
GSIZE = C // NUM_GROUPS            # 32 channels per group
EPS = 1e-6
SCALE = float(C) ** -0.5


class PatchedTileContext(tile.TileContext):
    """walrus in this container accepts only ONE sync-wait per instruction;
    split extra waits onto same-engine NoOps placed just before the
    instruction (same queue => waits still execute before it)."""

    def _lower_ordered_insts(self, ordered):
        for bb_name, insts in list(ordered.items()):
            new_list = []
            for inst in insts:
                si = inst.sync_info
                if si is not None and si.on_wait and len(si.on_wait) > 1:
                    waits = list(si.on_wait)
                    for w in waits[:-1]:
                        nop = mybir.InstNoOp(
                            name=self.nc.get_next_instruction_name(),
                            engine=inst.engine,
                            sync_info=mybir.SyncInfo(on_wait=[w], on_update=[]),
                            bass_nofuse=True,
                        )
                        new_list.append(nop)
                    si.on_wait = [waits[-1]]
                new_list.append(inst)
            ordered[bb_name] = new_list
        super()._lower_ordered_insts(ordered)

    def _drain_and_barrier(self, tick_clock, wait_clock):
        drain_inst = self.nc.sync.drain()
        wait_clock.add_sem_waits(
            drain_inst.ins, ScopedClock({None: tick_clock.global_clock})
        )
        si = drain_inst.ins.sync_info
        if si is not None and si.on_wait and len(si.on_wait) > 1:
            waits = list(si.on_wait)
            si.on_wait = [waits[0]]
            for w in waits[1:]:
                d2 = self.nc.sync.drain()
                d2.ins.sync_info = mybir.SyncInfo(on_wait=[w], on_update=[])
        self.nc.all_engine_barrier()
        assert self.sems is not None
        popped = self.nc._tile_sem_poison_stack.pop()
        assert popped is self._sem_poison
        self.nc.clear_and_free_semaphores(list(self.sems.allocated().values()))
        self.nc.all_engine_barrier()


def build_nc(reps=1):
    nc = bass.Bass(name=f"attnblk_r{reps}")

    xbf_d = nc.dram_tensor("xbf", [C, N], BF16, kind="ExternalInput")
    wkqtf8_d = nc.dram_tensor("wkqtf8", [P, 4 * 512], FP8, kind="ExternalInput")
    wpvtf8_d = nc.dram_tensor("wpvtf8", [P, 4 * 512], FP8, kind="ExternalInput")
    gamma_d = nc.dram_tensor("gamma", [C], F32, kind="ExternalInput")
    beta_d = nc.dram_tensor("beta", [C], F32, kind="ExternalInput")
    bu_d = nc.dram_tensor("bu", [C], F32, kind="ExternalInput")
    bpe_d = nc.dram_tensor("bpe", [C], F32, kind="ExternalInput")
    g4_d = nc.dram_tensor("g4", [P, 4], F32, kind="ExternalInput")
    g4t_d = nc.dram_tensor("g4t", [4, P], F32, kind="ExternalInput")
    out_d = nc.dram_tensor("out", [C, NQ], F32, kind="ExternalOutput")

    with PatchedTileContext(nc) as tc:
        with (
            tc.tile_pool(name="const", bufs=1) as const,
            tc.tile_pool(name="persist", bufs=1) as persist,
            tc.tile_pool(name="small", bufs=4) as small,
            tc.tile_pool(name="atp", bufs=36) as atp,
            tc.tile_pool(name="finp", bufs=6) as finp,
            tc.tile_pool(name="ps", bufs=1, space="PSUM") as ps,
        ):
            # ---------------- x load first: 3 DMA queues ----------------
            x_sb = persist.tile([P, CT, N], BF16)
            x_queues = [nc.sync, nc.scalar, nc.gpsimd]
            for piece in range(8):
                ct, xh = piece // 2, piece % 2
                x_queues[piece % 3].dma_start(
                    x_sb[:, ct, xh * 2048:(xh + 1) * 2048],
                    xbf_d[ct * P:(ct + 1) * P, xh * 2048:(xh + 1) * 2048],
                )

            # ---------------- constants (scalar queue, after x) --------------
            wkqt_f8 = const.tile([P, 2, 2, C], FP8)
            nc.gpsimd.dma_start(wkqt_f8[:], wkqtf8_d[:, :].rearrange("p (kp s co) -> p kp s co", kp=2, s=2))
            wpvt_f8 = const.tile([P, 2, 2, C], FP8)
            nc.gpsimd.dma_start(wpvt_f8[:], wpvtf8_d[:, :].rearrange("p (kp s co) -> p kp s co", kp=2, s=2))

            gam = const.tile([P, CT], F32)
            nc.sync.dma_start(gam[:], gamma_d[:].rearrange("(t p) -> p t", p=P))
            bet = const.tile([P, CT], F32)
            nc.sync.dma_start(bet[:], beta_d[:].rearrange("(t p) -> p t", p=P))
            bu4 = const.tile([P, CT], F32)
            nc.sync.dma_start(bu4[:], bu_d[:].rearrange("(t p) -> p t", p=P))
            bpe4 = const.tile([P, CT], F32)
            nc.sync.dma_start(bpe4[:], bpe_d[:].rearrange("(t p) -> p t", p=P))
            g4_sb = const.tile([P, 4], F32)
            nc.sync.dma_start(g4_sb[:], g4_d[:, :])
            g4t_sb = const.tile([4, P], F32)
            nc.sync.dma_start(g4t_sb[:], g4t_d[:, :])

            eps_sb = const.tile([P, 1], F32)
            nc.vector.memset(eps_sb[:], EPS)
            bias_m1 = const.tile([P, 1], F32)
            nc.vector.memset(bias_m1[:], -1.0)
            ones_f8 = const.tile([P, 2, P], FP8)
            nc.vector.memset(ones_f8[:], 1.0)

            # ---------------- persistent tensors ----------------
            hn_f8 = persist.tile([P, CT // 2, 2, N], FP8)
            vT_f8 = persist.tile([P, TT, 2, 512], FP8)
            u_f8 = persist.tile([P, CT // 2, 2, NQ], FP8)
            scale_sb = persist.tile([P, CT], F32)
            bias_sb = persist.tile([P, CT], F32)

            for _rep in range(reps):
                # ---------------- phase 0: groupnorm stats ------------------
                # ct0-2 on DVE (bn_stats), ct3 on ACT (wide sum/sumsq accum)
                # so the head-serial stats work runs on two engines.
                mrall = small.tile([4, 8], F32, tag="mrall")
                for ct in range(CT):
                    red = small.tile([P, 2], F32, tag="red", name=f"red_{ct}")
                    if ct == 3:
                        reds = small.tile([P, 4], F32, tag="reds", bufs=1)
                        sumsc = small.tile([P, 2048], F32, tag="sumsc", bufs=1)
                        for xh in range(2):
                            nc.scalar.activation(
                                sumsc[:], x_sb[:, ct, xh * 2048:(xh + 1) * 2048],
                                AF.Copy, accum_out=reds[:, xh:xh + 1],
                            )
                            nc.scalar.activation(
                                sumsc[:], x_sb[:, ct, xh * 2048:(xh + 1) * 2048],
                                AF.Square, accum_out=reds[:, 2 + xh:3 + xh],
                            )
                        nc.vector.tensor_tensor(red[:], reds[:, 0:4:2], reds[:, 1:4:2], ALU.add)
                        nc.vector.tensor_scalar_mul(red[:], red[:], 1.0 / N)
                    else:
                        bnst = small.tile([P, JC, 6], F32, tag="bnst", name=f"bnst_{ct}")
                        for jc in range(JC):
                            nc.vector.bn_stats(bnst[:, jc, :], x_sb[:, ct, jc * 512:(jc + 1) * 512])
                        mv = small.tile([P, 2], F32, tag="mv", name=f"mv_{ct}")
                        nc.vector.bn_aggr(mv[:], bnst[:])
                        msq = small.tile([P, 1], F32, tag="msq", name=f"msq_{ct}")
                        nc.scalar.activation(msq[:], mv[:, 0:1], AF.Square)
                        nc.scalar.copy(red[:, 0:1], mv[:, 0:1])
                        nc.vector.tensor_tensor(red[:, 1:2], mv[:, 1:2], msq[:], ALU.add)
                    gps = ps.tile([4, 2], F32, tag="pp", bufs=2, name=f"gps_{ct}")
                    nc.tensor.matmul(
                        gps[:], lhsT=g4_sb[:], rhs=red[:],
                        start=True, stop=True,
                    )
                    nc.scalar.copy(mrall[:, ct:ct + 1], gps[:, 0:1])
                    nc.scalar.copy(mrall[:, 4 + ct:5 + ct], gps[:, 1:2])
                # mu = mrall[:, :4]; var = mrall[:, 4:] - mu^2 (batched)
                musq = small.tile([4, 4], F32, tag="musq")
                nc.scalar.activation(musq[:], mrall[:, 0:4], AF.Square)
                var4 = small.tile([4, 4], F32, tag="var4")
                nc.vector.tensor_tensor(var4[:], mrall[:, 4:8], musq[:], ALU.subtract)
                std4 = small.tile([4, 4], F32, tag="std4")
                nc.scalar.activation(std4[:], var4[:], AF.Sqrt, bias=eps_sb[0:4, :])
                nc.vector.reciprocal(mrall[:, 4:8], std4[:])
                # one bcast matmul: [128, 8] = (mu | rstd) per channel
                mrp = ps.tile([P, 8], F32, tag="pp", bufs=2, name="mrp")
                nc.tensor.matmul(
                    mrp[:], lhsT=g4t_sb[:], rhs=mrall[:],
                    start=True, stop=True,
                )
                # scale = gamma * rstd ; bias = beta - mu * scale (batched)
                nc.vector.tensor_tensor(scale_sb[:], gam[:], mrp[:, 4:8], ALU.mult)
                tb = small.tile([P, 4], F32, tag="tb")
                nc.vector.tensor_tensor(tb[:], mrp[:, 0:4], scale_sb[:], ALU.mult)
                nc.vector.tensor_tensor(bias_sb[:], bet[:], tb[:], ALU.subtract)

                # ---------------- fused phase 1 + attention ----------------
                # Emission schedule: hn conversion (Pool) inline per jc;
                # vT' chains for jc<=5 inline, jc 6-7 deferred; den/attn@v/
                # normalize of chunk i drained as closures between the slices
                # of chunk i+1 so PE work spreads instead of bursting. u for
                # chunk i+1 is a drained closure too (pp ring is free of den
                # during slices since den accumulates post-loop).

                def hn_conv(jc):
                    for kc in range(CT):
                        nc.gpsimd.tensor_scalar(
                            hn_f8[:, kc // 2, kc % 2, jc * 512:(jc + 1) * 512],
                            x_sb[:, kc, jc * 512:(jc + 1) * 512],
                            scale_sb[:, kc:kc + 1], bias_sb[:, kc:kc + 1],
                            ALU.mult, ALU.add,
                        )

                def v_chains(jc, conv_eng):
                    for jl in range(4):
                        jt = jc * 4 + jl
                        pv = ps.tile([P, 512], F32, tag="pp", bufs=2, name=f"pv_{jc}_{jl}")
                        for kp in range(2):
                            nc.tensor.matmul(
                                pv[:], lhsT=hn_f8[:, kp, :, jt * P:(jt + 1) * P],
                                rhs=wpvt_f8[:, kp],
                                perf_mode=mybir.MatmulPerfMode.DoubleRow,
                                start=(kp == 0), stop=(kp == 1),
                            )
                        if conv_eng is nc.scalar:
                            nc.scalar.activation(vT_f8[:, jt // 2, jt % 2, :], pv[:], AF.Identity)
                        else:
                            nc.vector.tensor_copy(vT_f8[:, jt // 2, jt % 2, :], pv[:])

                def u_proj(ich):
                    # u = (Wk^T Wq) hn + bu for this query chunk; chunk 0's
                    # converts run on ACT (DVE is stats-busy at the head)
                    for co in range(CT):
                        pq = ps.tile([P, 512], F32, tag="pp", bufs=2, name=f"pq_{ich}_{co}")
                        for kp in range(2):
                            nc.tensor.matmul(
                                pq[:], lhsT=wkqt_f8[:, kp, :, co * P:(co + 1) * P],
                                rhs=hn_f8[:, kp, :, ich * 512:(ich + 1) * 512],
                                perf_mode=mybir.MatmulPerfMode.DoubleRow,
                                start=(kp == 0), stop=(kp == 1),
                            )
                        if ich == 0:
                            nc.scalar.activation(
                                u_f8[:, co // 2, co % 2, ich * 512:(ich + 1) * 512], pq[:],
                                AF.Identity, bias=bu4[:, co:co + 1],
                            )
                        else:
                            nc.vector.tensor_scalar(
                                u_f8[:, co // 2, co % 2, ich * 512:(ich + 1) * 512], pq[:],
                                bu4[:, co:co + 1], None, ALU.add,
                            )

                at2s = {ich: [] for ich in range(ICH)}
                rbs = {}

                def att_slice(ich, t):
                    # scores pair -> one exp (den accumulated post-loop)
                    sc = ps.tile([P, 2, 512], F32, tag="sc", bufs=2, name=f"sc_{ich}_{t}")
                    for s in range(2):
                        jt = 2 * t + s
                        for kp in range(2):
                            nc.tensor.matmul(
                                sc[:, s, :], lhsT=hn_f8[:, kp, :, jt * P:(jt + 1) * P],
                                rhs=u_f8[:, kp, :, ich * 512:(ich + 1) * 512],
                                perf_mode=mybir.MatmulPerfMode.DoubleRow,
                                start=(kp == 0), stop=(kp == 1),
                            )
                    at2 = atp.tile([P, 2, 512], FP8, tag="at", name=f"at2_{ich}_{t}")
                    at2s[ich].append(at2)
                    nc.scalar.activation(at2[:], sc[:], AF.Exp, scale=SCALE, bias=bias_m1[:])

                def den_rec(ich, psum_tag="pp"):
                    den = ps.tile([P, 512], F32, tag=psum_tag, bufs=2, name=f"den_{ich}")
                    for t in range(TT):
                        nc.tensor.matmul(
                            den[:], lhsT=ones_f8[:], rhs=at2s[ich][t][:],
                            perf_mode=mybir.MatmulPerfMode.DoubleRow,
                            start=(t == 0), stop=(t == TT - 1),
                        )
                    rb = finp.tile([P, 512], F32, tag="rb", name=f"rb_{ich}")
                    nc.vector.reciprocal(rb[:], den[:])
                    rbs[ich] = rb

                def av_chain(ich, ct, psum_tag="o2"):
                    o2t = ps.tile([P, 512], F32, tag=psum_tag, bufs=2, name=f"o2_{ich}_{ct}")
                    for t in range(TT):
                        nc.tensor.matmul(
                            o2t[:], lhsT=vT_f8[:, t, :, ct * P:(ct + 1) * P],
                            rhs=at2s[ich][t][:],
                            perf_mode=mybir.MatmulPerfMode.DoubleRow,
                            start=(t == 0), stop=(t == TT - 1),
                        )
                    t1 = finp.tile([P, 512], F32, tag="t1", name=f"t1_{ich}_{ct}")
                    nc.vector.tensor_tensor(t1[:], o2t[:], rbs[ich][:], ALU.mult)
                    fin = finp.tile([P, 512], F32, tag="fin", name=f"fin_{ich}_{ct}")
                    nc.vector.scalar_tensor_tensor(
                        fin[:], t1[:], bpe4[:, ct:ct + 1],
                        x_sb[:, ct, ich * 512:(ich + 1) * 512],
                        ALU.add, ALU.add,
                    )
                    out_q = nc.sync if ct % 2 == 0 else nc.scalar
                    out_q.dma_start(
                        out_d[ct * P:(ct + 1) * P, ich * 512:(ich + 1) * 512], fin[:]
                    )

                # --- chunk 0 window: phase 1 interleaved with its slices ---
                hn_conv(0)
                u_proj(0)
                v_chains(0, nc.scalar)
                for jc in range(1, JC):
                    hn_conv(jc)
                    if jc <= 5:
                        # one chain converted on ACT, three on DVE
                        v_chains(jc, nc.vector)
                    for t in (2 * (jc - 1), 2 * (jc - 1) + 1):
                        att_slice(0, t)
                for t in (2 * (JC - 1), 2 * (JC - 1) + 1):
                    att_slice(0, t)

                # --- chunks 1..3: slices with drained finish work ---
                pend = [lambda: v_chains(6, nc.vector),
                        lambda: v_chains(7, nc.scalar),
                        lambda: den_rec(0),
                        lambda: av_chain(0, 0), lambda: av_chain(0, 1),
                        lambda: u_proj(2),
                        lambda: av_chain(0, 2), lambda: av_chain(0, 3)]
                u_proj(1)
                last = ICH - 1
                tmaj = {}

                def tmaj_start(ct, upto):
                    o2t = ps.tile([P, 512], F32, tag="o2", bufs=2, name=f"o2_{last}_{ct}")
                    tmaj[ct] = o2t
                    for t in range(upto):
                        nc.tensor.matmul(
                            o2t[:], lhsT=vT_f8[:, t, :, ct * P:(ct + 1) * P],
                            rhs=at2s[last][t][:],
                            perf_mode=mybir.MatmulPerfMode.DoubleRow,
                            start=(t == 0), stop=False,
                        )

                for ich in range(1, ICH):
                    den_l = None
                    for t in range(TT):
                        att_slice(ich, t)
                        if ich == last:
                            if t == 4:
                                den_l = ps.tile([P, 512], F32, tag="pp", bufs=2,
                                                name=f"den_{last}")
                                for tp in range(5):
                                    nc.tensor.matmul(
                                        den_l[:], lhsT=ones_f8[:], rhs=at2s[last][tp][:],
                                        perf_mode=mybir.MatmulPerfMode.DoubleRow,
                                        start=(tp == 0), stop=False,
                                    )
                            elif t > 4:
                                nc.tensor.matmul(
                                    den_l[:], lhsT=ones_f8[:], rhs=at2s[last][t][:],
                                    perf_mode=mybir.MatmulPerfMode.DoubleRow,
                                    start=False, stop=(t == TT - 1),
                                )
                            if t == 12:
                                tmaj_start(0, 13)
                                tmaj_start(1, 13)
                            elif t > 12:
                                for ct in (0, 1):
                                    nc.tensor.matmul(
                                        tmaj[ct][:], lhsT=vT_f8[:, t, :, ct * P:(ct + 1) * P],
                                        rhs=at2s[last][t][:],
                                        perf_mode=mybir.MatmulPerfMode.DoubleRow,
                                        start=False, stop=(t == TT - 1),
                                    )
                        if t % 2 == 1 and pend:
                            pend.pop(0)()
                    while pend:
                        pend.pop(0)()
                    if ich < ICH - 1:
                        pend = [lambda i=ich: den_rec(i),
                                lambda i=ich: av_chain(i, 0),
                                lambda i=ich: av_chain(i, 1)]
                        if ich + 2 < ICH:
                            pend.append(lambda i=ich: u_proj(i + 2))
                        pend += [lambda i=ich: av_chain(i, 2),
                                 lambda i=ich: av_chain(i, 3)]
                # tail: reciprocal, finish t-major ct0/1, then ct2/3 on sc ring
                rb = finp.tile([P, 512], F32, tag="rb", name=f"rb_{last}")
                nc.vector.reciprocal(rb[:], den_l[:])
                rbs[last] = rb
                for ct in (0, 1):
                    t1 = finp.tile([P, 512], F32, tag="t1", name=f"t1_{last}_{ct}")
                    nc.vector.tensor_tensor(t1[:], tmaj[ct][:], rb[:], ALU.mult)
                    fin = finp.tile([P, 512], F32, tag="fin", name=f"fin_{last}_{ct}")
                    nc.vector.scalar_tensor_tensor(
                        fin[:], t1[:], bpe4[:, ct:ct + 1],
                        x_sb[:, ct, last * 512:(last + 1) * 512],
                        ALU.add, ALU.add,
                    )
                    out_q = nc.sync if ct % 2 == 0 else nc.scalar
                    out_q.dma_start(
                        out_d[ct * P:(ct + 1) * P, last * 512:(last + 1) * 512], fin[:]
                    )
                av_chain(last, 2, psum_tag="sc")
                av_chain(last, 3, psum_tag="sc")
    return nc


_NC = None


def _get_nc():
    global _NC
    if _NC is None:
        _NC = build_nc()
    return _NC


def _make_in_maps(x, gamma, beta, wq, bq, wk, bk, wv, bv, wp, bp):
    x = np.ascontiguousarray(np.asarray(x, dtype=np.float32)).reshape(4, C, N)
    bf = ml_dtypes.bfloat16

    def pack8(w):
        return np.ascontiguousarray(
            np.asarray(w, np.float32).T.reshape(2, 2, P, 512).transpose(2, 0, 1, 3)
            .reshape(P, 4 * 512).astype(mybir.dt.np(FP8))
        )

    g4i = np.zeros((P, 4), np.float32)
    for p in range(P):
        g4i[p, p // GSIZE] = 1.0
    g4 = g4i / GSIZE          # group-mean matmul (pre-scaled)
    g4t = np.ascontiguousarray(g4i.T)  # broadcast indicator (0/1)

    wq = np.asarray(wq, np.float32)
    wk = np.asarray(wk, np.float32)
    wv = np.asarray(wv, np.float32)
    wp = np.asarray(wp, np.float32)
    # scores fold: u = (Wk^T Wq) hn + Wk^T bq  (bk shift is softmax-invariant)
    wkq = wk.T @ wq
    bu = wk.T @ np.asarray(bq, np.float32)
    # output fold: attn@v with (Wp Wv) hn; bpe = Wp bv + bp
    wpv = wp @ wv
    bpe = np.asarray(bp, np.float32) + wp @ np.asarray(bv, np.float32)

    common = {
        "wkqtf8": pack8(wkq), "wpvtf8": pack8(wpv),
        "gamma": np.asarray(gamma, np.float32), "beta": np.asarray(beta, np.float32),
        "bu": np.ascontiguousarray(bu), "bpe": np.ascontiguousarray(bpe),
        "g4": g4, "g4t": g4t,
    }
    in_maps = []
    for core in range(8):
        bidx, half = core // 2, core % 2
        xb = x[bidx]
        if half == 0:
            xp = xb
        else:
            xp = np.concatenate([xb[:, NQ:], xb[:, :NQ]], axis=1)
        xp = np.ascontiguousarray(xp)
        in_maps.append({"xbf": xp.astype(bf), **common})
    return in_maps


def run(inputs, trace=False):
    nc = _get_nc()
    in_maps = _make_in_maps(**inputs)
    res = run_bass_kernel_spmd(nc, in_maps, list(range(8)), trace=trace)
    out = np.empty((4, C, N), np.float32)
    for core in range(8):
        bidx, half = core // 2, core % 2
        o = res.results[core]["out"]
        if half == 0:
            out[bidx, :, :NQ] = o
        else:
            out[bidx, :, NQ:] = o
    return out.reshape(4, C, 64, 64), res


def kernel(**inputs):
    out, _ = run(inputs, trace=False)
    return out


# revision 16
# speedup vs baseline: 1.0587x; 1.0137x over previous
"""Trainium2 Bass kernel for nn_AttnBlock (GroupNorm -> 1x1 q/k/v -> attention -> proj -> residual).

Input x: [4, 512, 64, 64] f32. Sharding: 8 cores = 4 batches x 2 query-halves.
Each core gets its batch's full x (columns permuted so its query half is first),
computes GroupNorm, attention over all 4096 keys for its 2048 queries, and
returns [512, 2048].

Weight-folding (exact algebra, done on host):
  scores^T[j,i] = (Wk hn_j + bk) . (Wq hn_i + bq)
               = hn_j . u_i + (const in j)   with u = (Wk^T Wq) hn + Wk^T bq
  The j-constant shifts every score of a query equally -> softmax-invariant,
  so the k projection, its converts, and bk disappear; scores contract the
  persistent fp8 hn directly against u.
  out = x + Wp(V attn + bv) + bp = x + Sum_j (Wp Wv hn_j) attn[j,:] + bpe
  with bpe = Wp bv + bp, so attn@v with vT' = (Wp Wv) hn directly produces
  projected output channels and the separate projection stage disappears.

Engine budget (cost model: ACT 0.833 ns/row +init, DVE 1.042 (0.5x all-SBUF
TensorScalar), Pool 0.833 SBUF-only, PE DR-fp8 107 ns per 512-row matmul):
  PE   ~72 us: stats mm + v'/u projections + scores + den + attn@v
  ACT  ~67 us: softmax exp only, two j-tiles per instruction ([P,2x512] PSUM)
  DVE  ~64 us: bn_stats, v'/u converts, o2*rb, reciprocal
  Pool ~21 us: hn fp8 conversion, fin = (o2*rb + bpe) + x, all SBUF-side
The denominator matmul uses an all-ones lhsT with M=128 so its PSUM result is
partition-broadcast; one DVE reciprocal yields the [128,512] normalizer,
applied to o2 before the (already folded) projection. Residual comes from the
resident bf16 x. GPSIMD cannot touch PSUM, so all PSUM readers are ACT/DVE.

Layouts (per core):
  x_sb  [128, 4, 4096]    bf16  (stats + hn source + residual)
  hn_f8 [128, 2, 2, 4096] fp8   c-pair-packed; lhsT for scores^T AND rhs for v'/u
  u_f8  [128, 2, 2, 2048] fp8   c-pair-packed rhs for scores^T
  vT_f8 [128, 16, 2, 512] fp8   j-pair-packed lhsT for attn@v (holds Wp Wv hn)
PSUM (8 banks): sc 2x[P,2,512] scores pairs + o2 2x[P,512] attn@v chains +
pp 2x[P,512] (stats mm, v'/u projections, den).
"""

import numpy as np
import ml_dtypes

import concourse.bass as bass
import concourse.mybir as mybir
import concourse.tile as tile
from concourse.vector_clock import ScopedClock
from concourse.bass_utils import run_bass_kernel_spmd

F32 = mybir.dt.float32
BF16 = mybir.dt.bfloat16
FP8 = mybir.dt.float8e4
AF = mybir.ActivationFunctionType
ALU = mybir.AluOpType

P = 128
C = 512          # channels
N = 4096         # spatial positions (64*64)
NQ = 2048        # queries per core (half)
CT = C // P      # 4 channel tiles
JC = N // 512    # 8 key chunks of 512
JT = N // P      # 32 key tiles of 128
TT = JT // 2     # 16 j-tile pairs
ICH = NQ // 512  # 4 query chunks of 512
NUM_GROUPS = 16
GSIZE = C // NUM_GROUPS            # 32 channels per group
EPS = 1e-6
SCALE = float(C) ** -0.5


class PatchedTileContext(tile.TileContext):
    """walrus in this container accepts only ONE sync-wait per instruction;
    split extra waits onto same-engine NoOps placed just before the
    instruction (same queue => waits still execute before it)."""

    def _lower_ordered_insts(self, ordered):
        for bb_name, insts in list(ordered.items()):
            new_list = []
            for inst in insts:
                si = inst.sync_info
                if si is not None and si.on_wait and len(si.on_wait) > 1:
                    waits = list(si.on_wait)
                    for w in waits[:-1]:
                        nop = mybir.InstNoOp(
                            name=self.nc.get_next_instruction_name(),
                            engine=inst.engine,
                            sync_info=mybir.SyncInfo(on_wait=[w], on_update=[]),
                            bass_nofuse=True,
                        )
                        new_list.append(nop)
                    si.on_wait = [waits[-1]]
                new_list.append(inst)
            ordered[bb_name] = new_list
        super()._lower_ordered_insts(ordered)

    def _drain_and_barrier(self, tick_clock, wait_clock):
        drain_inst = self.nc.sync.drain()
        wait_clock.add_sem_waits(
            drain_inst.ins, ScopedClock({None: tick_clock.global_clock})
        )
        si = drain_inst.ins.sync_info
        if si is not None and si.on_wait and len(si.on_wait) > 1:
            waits = list(si.on_wait)
            si.on_wait = [waits[0]]
            for w in waits[1:]:
                d2 = self.nc.sync.drain()
                d2.ins.sync_info = mybir.SyncInfo(on_wait=[w], on_update=[])
        self.nc.all_engine_barrier()
        assert self.sems is not None
        popped = self.nc._tile_sem_poison_stack.pop()
        assert popped is self._sem_poison
        self.nc.clear_and_free_semaphores(list(self.sems.allocated().values()))
        self.nc.all_engine_barrier()


def build_nc(reps=1):
    nc = bass.Bass(name=f"attnblk_r{reps}")

    xbf_d = nc.dram_tensor("xbf", [C, N], BF16, kind="ExternalInput")
    wkqtf8_d = nc.dram_tensor("wkqtf8", [P, 4 * 512], FP8, kind="ExternalInput")
    wpvtf8_d = nc.dram_tensor("wpvtf8", [P, 4 * 512], FP8, kind="ExternalInput")
    gamma_d = nc.dram_tensor("gamma", [C], F32, kind="ExternalInput")
    beta_d = nc.dram_tensor("beta", [C], F32, kind="ExternalInput")
    bu_d = nc.dram_tensor("bu", [C], F32, kind="ExternalInput")
    bpe_d = nc.dram_tensor("bpe", [C], F32, kind="ExternalInput")
    g4_d = nc.dram_tensor("g4", [P, 4], F32, kind="ExternalInput")
    g4t_d = nc.dram_tensor("g4t", [4, P], F32, kind="ExternalInput")
    out_d = nc.dram_tensor("out", [C, NQ], F32, kind="ExternalOutput")

    with PatchedTileContext(nc) as tc:
        with (
            tc.tile_pool(name="const", bufs=1) as const,
            tc.tile_pool(name="persist", bufs=1) as persist,
            tc.tile_pool(name="small", bufs=4) as small,
            tc.tile_pool(name="atp", bufs=36) as atp,
            tc.tile_pool(name="finp", bufs=6) as finp,
            tc.tile_pool(name="ps", bufs=1, space="PSUM") as ps,
        ):
            # ---------------- x load first: 3 DMA queues ----------------
            x_sb = persist.tile([P, CT, N], BF16)
            x_queues = [nc.sync, nc.scalar, nc.gpsimd]
            for piece in range(8):
                ct, xh = piece // 2, piece % 2
                x_queues[piece % 3].dma_start(
                    x_sb[:, ct, xh * 2048:(xh + 1) * 2048],
                    xbf_d[ct * P:(ct + 1) * P, xh * 2048:(xh + 1) * 2048],
                )

            # ---------------- constants (scalar queue, after x) --------------
            wkqt_f8 = const.tile([P, 2, 2, C], FP8)
            nc.gpsimd.dma_start(wkqt_f8[:], wkqtf8_d[:, :].rearrange("p (kp s co) -> p kp s co", kp=2, s=2))
            wpvt_f8 = const.tile([P, 2, 2, C], FP8)
            nc.gpsimd.dma_start(wpvt_f8[:], wpvtf8_d[:, :].rearrange("p (kp s co) -> p kp s co", kp=2, s=2))

            gam = const.tile([P, CT], F32)
            nc.sync.dma_start(gam[:], gamma_d[:].rearrange("(t p) -> p t", p=P))
            bet = const.tile([P, CT], F32)
            nc.sync.dma_start(bet[:], beta_d[:].rearrange("(t p) -> p t", p=P))
            bu4 = const.tile([P, CT], F32)
            nc.sync.dma_start(bu4[:], bu_d[:].rearrange("(t p) -> p t", p=P))
            bpe4 = const.tile([P, CT], F32)
            nc.sync.dma_start(bpe4[:], bpe_d[:].rearrange("(t p) -> p t", p=P))
            g4_sb = const.tile([P, 4], F32)
            nc.sync.dma_start(g4_sb[:], g4_d[:, :])
            g4t_sb = const.tile([4, P], F32)
            nc.sync.dma_start(g4t_sb[:], g4t_d[:, :])

            eps_sb = const.tile([P, 1], F32)
            nc.vector.memset(eps_sb[:], EPS)
            bias_m1 = const.tile([P, 1], F32)
            nc.vector.memset(bias_m1[:], -1.0)
            ones_f8 = const.tile([P, 2, P], FP8)
            nc.vector.memset(ones_f8[:], 1.0)

            # ---------------- persistent tensors ----------------
            hn_f8 = persist.tile([P, CT // 2, 2, N], FP8)
            vT_f8 = persist.tile([P, TT, 2, 512], FP8)
            u_f8 = persist.tile([P, CT // 2, 2, NQ], FP8)
            scale_sb = persist.tile([P, CT], F32)
            bias_sb = persist.tile([P, CT], F32)

            for _rep in range(reps):
                # ---------------- phase 0: groupnorm stats ------------------
                # ct0-2 on DVE (bn_stats), ct3 on ACT (wide sum/sumsq accum)
                # so the head-serial stats work runs on two engines.
                mrall = small.tile([4, 8], F32, tag="mrall")
                for ct in range(CT):
                    red = small.tile([P, 2], F32, tag="red", name=f"red_{ct}")
                    if ct == 3:
                        reds = small.tile([P, 4], F32, tag="reds", bufs=1)
                        sumsc = small.tile([P, 2048], F32, tag="sumsc", bufs=1)
                        for xh in range(2):
                            nc.scalar.activation(
                                sumsc[:], x_sb[:, ct, xh * 2048:(xh + 1) * 2048],
                                AF.Copy, accum_out=reds[:, xh:xh + 1],
                            )
                            nc.scalar.activation(
                                sumsc[:], x_sb[:, ct, xh * 2048:(xh + 1) * 2048],
                                AF.Square, accum_out=reds[:, 2 + xh:3 + xh],
                            )
                        nc.vector.tensor_tensor(red[:], reds[:, 0:4:2], reds[:, 1:4:2], ALU.add)
                        nc.vector.tensor_scalar_mul(red[:], red[:], 1.0 / N)
                    else:
                        bnst = small.tile([P, JC, 6], F32, tag="bnst", name=f"bnst_{ct}")
                        for jc in range(JC):
                            nc.vector.bn_stats(bnst[:, jc, :], x_sb[:, ct, jc * 512:(jc + 1) * 512])
                        mv = small.tile([P, 2], F32, tag="mv", name=f"mv_{ct}")
                        nc.vector.bn_aggr(mv[:], bnst[:])
                        msq = small.tile([P, 1], F32, tag="msq", name=f"msq_{ct}")
                        nc.scalar.activation(msq[:], mv[:, 0:1], AF.Square)
                        nc.scalar.copy(red[:, 0:1], mv[:, 0:1])
                        nc.vector.tensor_tensor(red[:, 1:2], mv[:, 1:2], msq[:], ALU.add)
                    gps = ps.tile([4, 2], F32, tag="pp", bufs=2, name=f"gps_{ct}")
                    nc.tensor.matmul(
                        gps[:], lhsT=g4_sb[:], rhs=red[:],
                        start=True, stop=True,
                    )
                    nc.scalar.copy(mrall[:, ct:ct + 1], gps[:, 0:1])
                    nc.scalar.copy(mrall[:, 4 + ct:5 + ct], gps[:, 1:2])
                # mu = mrall[:, :4]; var = mrall[:, 4:] - mu^2 (batched)
                musq = small.tile([4, 4], F32, tag="musq")
                nc.scalar.activation(musq[:], mrall[:, 0:4], AF.Square)
                var4 = small.tile([4, 4], F32, tag="var4")
                nc.vector.tensor_tensor(var4[:], mrall[:, 4:8], musq[:], ALU.subtract)
                std4 = small.tile([4, 4], F32, tag="std4")
                nc.scalar.activation(std4[:], var4[:], AF.Sqrt, bias=eps_sb[0:4, :])
                nc.vector.reciprocal(mrall[:, 4:8], std4[:])
                # one bcast matmul: [128, 8] = (mu | rstd) per channel
                mrp = ps.tile([P, 8], F32, tag="pp", bufs=2, name="mrp")
                nc.tensor.matmul(
                    mrp[:], lhsT=g4t_sb[:], rhs=mrall[:],
                    start=True, stop=True,
                )
                # scale = gamma * rstd ; bias = beta - mu * scale (batched)
                nc.vector.tensor_tensor(scale_sb[:], gam[:], mrp[:, 4:8], ALU.mult)
                tb = small.tile([P, 4], F32, tag="tb")
                nc.vector.tensor_tensor(tb[:], mrp[:, 0:4], scale_sb[:], ALU.mult)
                nc.vector.tensor_tensor(bias_sb[:], bet[:], tb[:], ALU.subtract)

                # ---------------- fused phase 1 + attention ----------------
                # Emission schedule: hn conversion (Pool) inline per jc;
                # vT' chains for jc<=5 inline, jc 6-7 deferred; den/attn@v/
                # normalize of chunk i drained as closures between the slices
                # of chunk i+1 so PE work spreads instead of bursting. u for
                # chunk i+1 is a drained closure too (pp ring is free of den
                # during slices since den accumulates post-loop).

                def hn_conv(jc):
                    for kc in range(CT):
                        nc.gpsimd.tensor_scalar(
                            hn_f8[:, kc // 2, kc % 2, jc * 512:(jc + 1) * 512],
                            x_sb[:, kc, jc * 512:(jc + 1) * 512],
                            scale_sb[:, kc:kc + 1], bias_sb[:, kc:kc + 1],
                            ALU.mult, ALU.add,
                        )

                def v_chains(jc, conv_eng):
                    for jl in range(4):
                        jt = jc * 4 + jl
                        pv = ps.tile([P, 512], F32, tag="pp", bufs=2, name=f"pv_{jc}_{jl}")
                        for kp in range(2):
                            nc.tensor.matmul(
                                pv[:], lhsT=hn_f8[:, kp, :, jt * P:(jt + 1) * P],
                                rhs=wpvt_f8[:, kp],
                                perf_mode=mybir.MatmulPerfMode.DoubleRow,
                                start=(kp == 0), stop=(kp == 1),
                            )
                        if conv_eng is nc.scalar:
                            nc.scalar.activation(vT_f8[:, jt // 2, jt % 2, :], pv[:], AF.Identity)
                        else:
                            nc.vector.tensor_copy(vT_f8[:, jt // 2, jt % 2, :], pv[:])

                def u_proj(ich):
                    # u = (Wk^T Wq) hn + bu for this query chunk; chunk 0's
                    # converts run on ACT (DVE is stats-busy at the head)
                    for co in range(CT):
                        pq = ps.tile([P, 512], F32, tag="pp", bufs=2, name=f"pq_{ich}_{co}")
                        for kp in range(2):
                            nc.tensor.matmul(
                                pq[:], lhsT=wkqt_f8[:, kp, :, co * P:(co + 1) * P],
                                rhs=hn_f8[:, kp, :, ich * 512:(ich + 1) * 512],
                                perf_mode=mybir.MatmulPerfMode.DoubleRow,
                                start=(kp == 0), stop=(kp == 1),
                            )
                        if ich == 0:
                            nc.scalar.activation(
                                u_f8[:, co // 2, co % 2, ich * 512:(ich + 1) * 512], pq[:],
                                AF.Identity, bias=bu4[:, co:co + 1],
                            )
                        else:
                            nc.vector.tensor_scalar(
                                u_f8[:, co // 2, co % 2, ich * 512:(ich + 1) * 512], pq[:],
                                bu4[:, co:co + 1], None, ALU.add,
                            )

                at2s = {ich: [] for ich in range(ICH)}
                rbs = {}

                def att_slice(ich, t):
                    # scores pair -> one exp (den accumulated post-loop)
                    sc = ps.tile([P, 2, 512], F32, tag="sc", bufs=2, name=f"sc_{ich}_{t}")
                    for s in range(2):
                        jt = 2 * t + s
                        for kp in range(2):
                            nc.tensor.matmul(
                                sc[:, s, :], lhsT=hn_f8[:, kp, :, jt * P:(jt + 1) * P],
                                rhs=u_f8[:, kp, :, ich * 512:(ich + 1) * 512],
                                perf_mode=mybir.MatmulPerfMode.DoubleRow,
                                start=(kp == 0), stop=(kp == 1),
                            )
                    at2 = atp.tile([P, 2, 512], FP8, tag="at", name=f"at2_{ich}_{t}")
                    at2s[ich].append(at2)
                    nc.scalar.activation(at2[:], sc[:], AF.Exp, scale=SCALE, bias=bias_m1[:])

                def den_rec(ich, psum_tag="pp"):
                    den = ps.tile([P, 512], F32, tag=psum_tag, bufs=2, name=f"den_{ich}")
                    for t in range(TT):
                        nc.tensor.matmul(
                            den[:], lhsT=ones_f8[:], rhs=at2s[ich][t][:],
                            perf_mode=mybir.MatmulPerfMode.DoubleRow,
                            start=(t == 0), stop=(t == TT - 1),
                        )
                    rb = finp.tile([P, 512], F32, tag="rb", name=f"rb_{ich}")
                    nc.vector.reciprocal(rb[:], den[:])
                    rbs[ich] = rb

                def av_chain(ich, ct, psum_tag="o2"):
                    o2t = ps.tile([P, 512], F32, tag=psum_tag, bufs=2, name=f"o2_{ich}_{ct}")
                    for t in range(TT):
                        nc.tensor.matmul(
                            o2t[:], lhsT=vT_f8[:, t, :, ct * P:(ct + 1) * P],
                            rhs=at2s[ich][t][:],
                            perf_mode=mybir.MatmulPerfMode.DoubleRow,
                            start=(t == 0), stop=(t == TT - 1),
                        )
                    t1 = finp.tile([P, 512], F32, tag="t1", name=f"t1_{ich}_{ct}")
                    nc.vector.tensor_tensor(t1[:], o2t[:], rbs[ich][:], ALU.mult)
                    fin = finp.tile([P, 512], F32, tag="fin", name=f"fin_{ich}_{ct}")
                    nc.vector.scalar_tensor_tensor(
                        fin[:], t1[:], bpe4[:, ct:ct + 1],
                        x_sb[:, ct, ich * 512:(ich + 1) * 512],
                        ALU.add, ALU.add,
                    )
                    out_q = nc.sync if ct % 2 == 0 else nc.scalar
                    out_q.dma_start(
                        out_d[ct * P:(ct + 1) * P, ich * 512:(ich + 1) * 512], fin[:]
                    )

                # --- chunk 0 window: phase 1 interleaved with its slices ---
                hn_conv(0)
                u_proj(0)
                v_chains(0, nc.vector)
                for jc in range(1, JC):
                    hn_conv(jc)
                    if jc <= 5:
                        # one chain converted on ACT, three on DVE
                        v_chains(jc, nc.vector)
                    for t in (2 * (jc - 1), 2 * (jc - 1) + 1):
                        att_slice(0, t)
                for t in (2 * (JC - 1), 2 * (JC - 1) + 1):
                    att_slice(0, t)

                # --- chunks 1..3: slices with drained finish work ---
                pend = [lambda: v_chains(6, nc.vector),
                        lambda: v_chains(7, nc.scalar),
                        lambda: den_rec(0),
                        lambda: av_chain(0, 0), lambda: av_chain(0, 1),
                        lambda: u_proj(2),
                        lambda: av_chain(0, 2), lambda: av_chain(0, 3)]
                u_proj(1)
                last = ICH - 1
                tmaj = {}

                def tmaj_start(ct, upto):
                    o2t = ps.tile([P, 512], F32, tag="o2", bufs=2, name=f"o2_{last}_{ct}")
                    tmaj[ct] = o2t
                    for t in range(upto):
                        nc.tensor.matmul(
                            o2t[:], lhsT=vT_f8[:, t, :, ct * P:(ct + 1) * P],
                            rhs=at2s[last][t][:],
                            perf_mode=mybir.MatmulPerfMode.DoubleRow,
                            start=(t == 0), stop=False,
                        )

                for ich in range(1, ICH):
                    den_l = None
                    for t in range(TT):
                        att_slice(ich, t)
                        if ich == last:
                            if t == 4:
                                den_l = ps.tile([P, 512], F32, tag="pp", bufs=2,
                                                name=f"den_{last}")
                                for tp in range(5):
                                    nc.tensor.matmul(
                                        den_l[:], lhsT=ones_f8[:], rhs=at2s[last][tp][:],
                                        perf_mode=mybir.MatmulPerfMode.DoubleRow,
                                        start=(tp == 0), stop=False,
                                    )
                            elif t > 4:
                                nc.tensor.matmul(
                                    den_l[:], lhsT=ones_f8[:], rhs=at2s[last][t][:],
                                    perf_mode=mybir.MatmulPerfMode.DoubleRow,
                                    start=False, stop=(t == TT - 1),
                                )
                            if t == 12:
                                tmaj_start(0, 13)
                                tmaj_start(1, 13)
                            elif t > 12:
                                for ct in (0, 1):
                                    nc.tensor.matmul(
                                        tmaj[ct][:], lhsT=vT_f8[:, t, :, ct * P:(ct + 1) * P],
                                        rhs=at2s[last][t][:],
                                        perf_mode=mybir.MatmulPerfMode.DoubleRow,
                                        start=False, stop=(t == TT - 1),
                                    )
                        if t % 2 == 1 and pend:
                            pend.pop(0)()
                    while pend:
                        pend.pop(0)()
                    if ich < ICH - 1:
                        pend = [lambda i=ich: den_rec(i),
                                lambda i=ich: av_chain(i, 0),
                                lambda i=ich: av_chain(i, 1)]
                        if ich + 2 < ICH:
                            pend.append(lambda i=ich: u_proj(i + 2))
                        pend += [lambda i=ich: av_chain(i, 2),
                                 lambda i=ich: av_chain(i, 3)]
                # tail: reciprocal, finish t-major ct0/1, then ct2/3 on sc ring
                rb = finp.tile([P, 512], F32, tag="rb", name=f"rb_{last}")
                nc.vector.reciprocal(rb[:], den_l[:])
                rbs[last] = rb
                for ct in (0, 1):
                    t1 = finp.tile([P, 512], F32, tag="t1", name=f"t1_{last}_{ct}")
                    nc.vector.tensor_tensor(t1[:], tmaj[ct][:], rb[:], ALU.mult)
                    fin = finp.tile([P, 512], F32, tag="fin", name=f"fin_{last}_{ct}")
                    nc.vector.scalar_tensor_tensor(
                        fin[:], t1[:], bpe4[:, ct:ct + 1],
                        x_sb[:, ct, last * 512:(last + 1) * 512],
                        ALU.add, ALU.add,
                    )
                    out_q = nc.sync if ct % 2 == 0 else nc.scalar
                    out_q.dma_start(
                        out_d[ct * P:(ct + 1) * P, last * 512:(last + 1) * 512], fin[:]
                    )
                av_chain(last, 2, psum_tag="sc")
                av_chain(last, 3, psum_tag="sc")
    return nc


_NC = None


def _get_nc():
    global _NC
    if _NC is None:
        _NC = build_nc()
    return _NC


def _make_in_maps(x, gamma, beta, wq, bq, wk, bk, wv, bv, wp, bp):
    x = np.ascontiguousarray(np.asarray(x, dtype=np.float32)).reshape(4, C, N)
    bf = ml_dtypes.bfloat16

    def pack8(w):
        return np.ascontiguousarray(
            np.asarray(w, np.float32).T.reshape(2, 2, P, 512).transpose(2, 0, 1, 3)
            .reshape(P, 4 * 512).astype(mybir.dt.np(FP8))
        )

    g4i = np.zeros((P, 4), np.float32)
    for p in range(P):
        g4i[p, p // GSIZE] = 1.0
    g4 = g4i / GSIZE          # group-mean matmul (pre-scaled)
    g4t = np.ascontiguousarray(g4i.T)  # broadcast indicator (0/1)

    wq = np.asarray(wq, np.float32)
    wk = np.asarray(wk, np.float32)
    wv = np.asarray(wv, np.float32)
    wp = np.asarray(wp, np.float32)
    # scores fold: u = (Wk^T Wq) hn + Wk^T bq  (bk shift is softmax-invariant)
    wkq = wk.T @ wq
    bu = wk.T @ np.asarray(bq, np.float32)
    # output fold: attn@v with (Wp Wv) hn; bpe = Wp bv + bp
    wpv = wp @ wv
    bpe = np.asarray(bp, np.float32) + wp @ np.asarray(bv, np.float32)

    common = {
        "wkqtf8": pack8(wkq), "wpvtf8": pack8(wpv),
        "gamma": np.asarray(gamma, np.float32), "beta": np.asarray(beta, np.float32),
        "bu": np.ascontiguousarray(bu), "bpe": np.ascontiguousarray(bpe),
        "g4": g4, "g4t": g4t,
    }
    in_maps = []
    for core in range(8):
        bidx, half = core // 2, core % 2
        xb = x[bidx]
        if half == 0:
            xp = xb
        else:
            xp = np.concatenate([xb[:, NQ:], xb[:, :NQ]], axis=1)
        xp = np.ascontiguousarray(xp)
        in_maps.append({"xbf": xp.astype(bf), **common})
    return in_maps


def run(inputs, trace=False):
    nc = _get_nc()
    in_maps = _make_in_maps(**inputs)
    res = run_bass_kernel_spmd(nc, in_maps, list(range(8)), trace=trace)
    out = np.empty((4, C, N), np.float32)
    for core in range(8):
        bidx, half = core // 2, core % 2
        o = res.results[core]["out"]
        if half == 0:
            out[bidx, :, :NQ] = o
        else:
            out[bidx, :, NQ:] = o
    return out.reshape(4, C, 64, 64), res


def kernel(**inputs):
    out, _ = run(inputs, trace=False)
    return out


# revision 17
# speedup vs baseline: 1.0698x; 1.0105x over previous
"""Trainium2 Bass kernel for nn_AttnBlock (GroupNorm -> 1x1 q/k/v -> attention -> proj -> residual).

Input x: [4, 512, 64, 64] f32. Sharding: 8 cores = 4 batches x 2 query-halves.
Each core gets its batch's full x (columns permuted so its query half is first),
computes GroupNorm, attention over all 4096 keys for its 2048 queries, and
returns [512, 2048].

Weight-folding (exact algebra, done on host):
  scores^T[j,i] = (Wk hn_j + bk) . (Wq hn_i + bq)
               = hn_j . u_i + (const in j)   with u = (Wk^T Wq) hn + Wk^T bq
  The j-constant shifts every score of a query equally -> softmax-invariant,
  so the k projection, its converts, and bk disappear; scores contract the
  persistent fp8 hn directly against u.
  out = x + Wp(V attn + bv) + bp = x + Sum_j (Wp Wv hn_j) attn[j,:] + bpe
  with bpe = Wp bv + bp, so attn@v with vT' = (Wp Wv) hn directly produces
  projected output channels and the separate projection stage disappears.

Engine budget (cost model: ACT 0.833 ns/row +init, DVE 1.042 (0.5x all-SBUF
TensorScalar), Pool 0.833 SBUF-only, PE DR-fp8 107 ns per 512-row matmul):
  PE   ~72 us: stats mm + v'/u projections + scores + den + attn@v
  ACT  ~67 us: softmax exp only, two j-tiles per instruction ([P,2x512] PSUM)
  DVE  ~64 us: bn_stats, v'/u converts, o2*rb, reciprocal
  Pool ~21 us: hn fp8 conversion, fin = (o2*rb + bpe) + x, all SBUF-side
The denominator matmul uses an all-ones lhsT with M=128 so its PSUM result is
partition-broadcast; one DVE reciprocal yields the [128,512] normalizer,
applied to o2 before the (already folded) projection. Residual comes from the
resident bf16 x. GPSIMD cannot touch PSUM, so all PSUM readers are ACT/DVE.

Layouts (per core):
  x_sb  [128, 4, 4096]    bf16  (stats + hn source + residual)
  hn_f8 [128, 2, 2, 4096] fp8   c-pair-packed; lhsT for scores^T AND rhs for v'/u
  u_f8  [128, 2, 2, 2048] fp8   c-pair-packed rhs for scores^T
  vT_f8 [128, 16, 2, 512] fp8   j-pair-packed lhsT for attn@v (holds Wp Wv hn)
PSUM (8 banks): sc 2x[P,2,512] scores pairs + o2 2x[P,512] attn@v chains +
pp 2x[P,512] (stats mm, v'/u projections, den).
"""

import numpy as np
import ml_dtypes

import concourse.bass as bass
import concourse.mybir as mybir
import concourse.tile as tile
from concourse.vector_clock import ScopedClock
from concourse.bass_utils import run_bass_kernel_spmd

F32 = mybir.dt.float32
BF16 = mybir.dt.bfloat16
FP8 = mybir.dt.float8e4
AF = mybir.ActivationFunctionType
ALU = mybir.AluOpType

P = 128
C = 512          # channels
N = 4096         # spatial positions (64*64)
NQ = 2048        # queries per core (half)
CT = C // P      # 4 channel tiles
JC = N // 512    # 8 key chunks of 512
JT = N // P      # 32 key tiles of 128
TT = JT // 2     # 16 j-tile pairs
ICH = NQ // 512  # 4 query chunks of 512
NUM_GROUPS = 16
GSIZE = C // NUM_GROUPS            # 32 channels per group
EPS = 1e-6
SCALE = float(C) ** -0.5


class PatchedTileContext(tile.TileContext):
    """walrus in this container accepts only ONE sync-wait per instruction;
    split extra waits onto same-engine NoOps placed just before the
    instruction (same queue => waits still execute before it)."""

    def _lower_ordered_insts(self, ordered):
        for bb_name, insts in list(ordered.items()):
            new_list = []
            for inst in insts:
                si = inst.sync_info
                if si is not None and si.on_wait and len(si.on_wait) > 1:
                    waits = list(si.on_wait)
                    for w in waits[:-1]:
                        nop = mybir.InstNoOp(
                            name=self.nc.get_next_instruction_name(),
                            engine=inst.engine,
                            sync_info=mybir.SyncInfo(on_wait=[w], on_update=[]),
                            bass_nofuse=True,
                        )
                        new_list.append(nop)
                    si.on_wait = [waits[-1]]
                new_list.append(inst)
            ordered[bb_name] = new_list
        super()._lower_ordered_insts(ordered)

    def _drain_and_barrier(self, tick_clock, wait_clock):
        drain_inst = self.nc.sync.drain()
        wait_clock.add_sem_waits(
            drain_inst.ins, ScopedClock({None: tick_clock.global_clock})
        )
        si = drain_inst.ins.sync_info
        if si is not None and si.on_wait and len(si.on_wait) > 1:
            waits = list(si.on_wait)
            si.on_wait = [waits[0]]
            for w in waits[1:]:
                d2 = self.nc.sync.drain()
                d2.ins.sync_info = mybir.SyncInfo(on_wait=[w], on_update=[])
        self.nc.all_engine_barrier()
        assert self.sems is not None
        popped = self.nc._tile_sem_poison_stack.pop()
        assert popped is self._sem_poison
        self.nc.clear_and_free_semaphores(list(self.sems.allocated().values()))
        self.nc.all_engine_barrier()


def build_nc(reps=1):
    nc = bass.Bass(name=f"attnblk_r{reps}")

    xbf_d = nc.dram_tensor("xbf", [C, N], BF16, kind="ExternalInput")
    wkqtf8_d = nc.dram_tensor("wkqtf8", [P, 4 * 512], FP8, kind="ExternalInput")
    wpvtf8_d = nc.dram_tensor("wpvtf8", [P, 4 * 512], FP8, kind="ExternalInput")
    gamma_d = nc.dram_tensor("gamma", [C], F32, kind="ExternalInput")
    beta_d = nc.dram_tensor("beta", [C], F32, kind="ExternalInput")
    bu_d = nc.dram_tensor("bu", [C], F32, kind="ExternalInput")
    bpe_d = nc.dram_tensor("bpe", [C], F32, kind="ExternalInput")
    g4_d = nc.dram_tensor("g4", [P, 4], F32, kind="ExternalInput")
    g4t_d = nc.dram_tensor("g4t", [4, P], F32, kind="ExternalInput")
    out_d = nc.dram_tensor("out", [C, NQ], F32, kind="ExternalOutput")

    with PatchedTileContext(nc) as tc:
        with (
            tc.tile_pool(name="const", bufs=1) as const,
            tc.tile_pool(name="persist", bufs=1) as persist,
            tc.tile_pool(name="small", bufs=4) as small,
            tc.tile_pool(name="atp", bufs=36) as atp,
            tc.tile_pool(name="finp", bufs=6) as finp,
            tc.tile_pool(name="ps", bufs=1, space="PSUM") as ps,
        ):
            # ---------------- x load first: 3 DMA queues ----------------
            x_sb = persist.tile([P, CT, N], BF16)
            x_queues = [nc.sync, nc.scalar, nc.gpsimd]
            for piece in range(8):
                ct, xh = piece // 2, piece % 2
                x_queues[piece % 3].dma_start(
                    x_sb[:, ct, xh * 2048:(xh + 1) * 2048],
                    xbf_d[ct * P:(ct + 1) * P, xh * 2048:(xh + 1) * 2048],
                )

            # ---------------- constants (scalar queue, after x) --------------
            wkqt_f8 = const.tile([P, 2, 2, C], FP8)
            nc.gpsimd.dma_start(wkqt_f8[:], wkqtf8_d[:, :].rearrange("p (kp s co) -> p kp s co", kp=2, s=2))
            wpvt_f8 = const.tile([P, 2, 2, C], FP8)
            nc.gpsimd.dma_start(wpvt_f8[:], wpvtf8_d[:, :].rearrange("p (kp s co) -> p kp s co", kp=2, s=2))

            gam = const.tile([P, CT], F32)
            nc.sync.dma_start(gam[:], gamma_d[:].rearrange("(t p) -> p t", p=P))
            bet = const.tile([P, CT], F32)
            nc.sync.dma_start(bet[:], beta_d[:].rearrange("(t p) -> p t", p=P))
            bu4 = const.tile([P, CT], F32)
            nc.sync.dma_start(bu4[:], bu_d[:].rearrange("(t p) -> p t", p=P))
            bpe4 = const.tile([P, CT], F32)
            nc.sync.dma_start(bpe4[:], bpe_d[:].rearrange("(t p) -> p t", p=P))
            g4_sb = const.tile([P, 4], F32)
            nc.sync.dma_start(g4_sb[:], g4_d[:, :])
            g4t_sb = const.tile([4, P], F32)
            nc.sync.dma_start(g4t_sb[:], g4t_d[:, :])

            eps_sb = const.tile([P, 1], F32)
            nc.vector.memset(eps_sb[:], EPS)
            bias_m1 = const.tile([P, 1], F32)
            nc.vector.memset(bias_m1[:], -1.0)
            ones_f8 = const.tile([P, 2, P], FP8)
            nc.vector.memset(ones_f8[:], 1.0)

            # ---------------- persistent tensors ----------------
            hn_f8 = persist.tile([P, CT // 2, 2, N], FP8)
            vT_f8 = persist.tile([P, TT, 2, 512], FP8)
            u_f8 = persist.tile([P, CT // 2, 2, NQ], FP8)
            scale_sb = persist.tile([P, CT], F32)
            bias_sb = persist.tile([P, CT], F32)

            for _rep in range(reps):
                # ---------------- phase 0: groupnorm stats ------------------
                # ct0-2 on DVE (bn_stats), ct3 on ACT (wide sum/sumsq accum)
                # so the head-serial stats work runs on two engines.
                mrall = small.tile([4, 8], F32, tag="mrall")
                for ct in range(CT):
                    red = small.tile([P, 2], F32, tag="red", name=f"red_{ct}")
                    if ct == 3:
                        reds = small.tile([P, 4], F32, tag="reds", bufs=1)
                        sumsc = small.tile([P, 2048], F32, tag="sumsc", bufs=1)
                        for xh in range(2):
                            nc.scalar.activation(
                                sumsc[:], x_sb[:, ct, xh * 2048:(xh + 1) * 2048],
                                AF.Copy, accum_out=reds[:, xh:xh + 1],
                            )
                            nc.scalar.activation(
                                sumsc[:], x_sb[:, ct, xh * 2048:(xh + 1) * 2048],
                                AF.Square, accum_out=reds[:, 2 + xh:3 + xh],
                            )
                        nc.vector.tensor_tensor(red[:], reds[:, 0:4:2], reds[:, 1:4:2], ALU.add)
                        nc.vector.tensor_scalar_mul(red[:], red[:], 1.0 / N)
                    else:
                        bnst = small.tile([P, JC, 6], F32, tag="bnst", name=f"bnst_{ct}")
                        for jc in range(JC):
                            nc.vector.bn_stats(bnst[:, jc, :], x_sb[:, ct, jc * 512:(jc + 1) * 512])
                        mv = small.tile([P, 2], F32, tag="mv", name=f"mv_{ct}")
                        nc.vector.bn_aggr(mv[:], bnst[:])
                        msq = small.tile([P, 1], F32, tag="msq", name=f"msq_{ct}")
                        nc.vector.tensor_tensor(msq[:], mv[:, 0:1], mv[:, 0:1], ALU.mult)
                        nc.vector.tensor_copy(red[:, 0:1], mv[:, 0:1])
                        nc.vector.tensor_tensor(red[:, 1:2], mv[:, 1:2], msq[:], ALU.add)
                    gps = ps.tile([4, 2], F32, tag="pp", bufs=2, name=f"gps_{ct}")
                    nc.tensor.matmul(
                        gps[:], lhsT=g4_sb[:], rhs=red[:],
                        start=True, stop=True,
                    )
                    nc.scalar.copy(mrall[:, ct:ct + 1], gps[:, 0:1])
                    nc.scalar.copy(mrall[:, 4 + ct:5 + ct], gps[:, 1:2])
                # mu = mrall[:, :4]; var = mrall[:, 4:] - mu^2 (batched)
                musq = small.tile([4, 4], F32, tag="musq")
                nc.scalar.activation(musq[:], mrall[:, 0:4], AF.Square)
                var4 = small.tile([4, 4], F32, tag="var4")
                nc.vector.tensor_tensor(var4[:], mrall[:, 4:8], musq[:], ALU.subtract)
                std4 = small.tile([4, 4], F32, tag="std4")
                nc.scalar.activation(std4[:], var4[:], AF.Sqrt, bias=eps_sb[0:4, :])
                nc.vector.reciprocal(mrall[:, 4:8], std4[:])
                # one bcast matmul: [128, 8] = (mu | rstd) per channel
                mrp = ps.tile([P, 8], F32, tag="pp", bufs=2, name="mrp")
                nc.tensor.matmul(
                    mrp[:], lhsT=g4t_sb[:], rhs=mrall[:],
                    start=True, stop=True,
                )
                # scale = gamma * rstd ; bias = beta - mu * scale (batched)
                nc.vector.tensor_tensor(scale_sb[:], gam[:], mrp[:, 4:8], ALU.mult)
                tb = small.tile([P, 4], F32, tag="tb")
                nc.vector.tensor_tensor(tb[:], mrp[:, 0:4], scale_sb[:], ALU.mult)
                nc.vector.tensor_tensor(bias_sb[:], bet[:], tb[:], ALU.subtract)

                # ---------------- fused phase 1 + attention ----------------
                # Emission schedule: hn conversion (Pool) inline per jc;
                # vT' chains for jc<=5 inline, jc 6-7 deferred; den/attn@v/
                # normalize of chunk i drained as closures between the slices
                # of chunk i+1 so PE work spreads instead of bursting. u for
                # chunk i+1 is a drained closure too (pp ring is free of den
                # during slices since den accumulates post-loop).

                def hn_conv(jc, split=False):
                    for kc in range(CT):
                        eng = nc.vector if (split and kc % 2 == 1) else nc.gpsimd
                        eng.tensor_scalar(
                            hn_f8[:, kc // 2, kc % 2, jc * 512:(jc + 1) * 512],
                            x_sb[:, kc, jc * 512:(jc + 1) * 512],
                            scale_sb[:, kc:kc + 1], bias_sb[:, kc:kc + 1],
                            ALU.mult, ALU.add,
                        )

                def v_chains(jc, conv_eng):
                    for jl in range(4):
                        jt = jc * 4 + jl
                        pv = ps.tile([P, 512], F32, tag="pp", bufs=2, name=f"pv_{jc}_{jl}")
                        for kp in range(2):
                            nc.tensor.matmul(
                                pv[:], lhsT=hn_f8[:, kp, :, jt * P:(jt + 1) * P],
                                rhs=wpvt_f8[:, kp],
                                perf_mode=mybir.MatmulPerfMode.DoubleRow,
                                start=(kp == 0), stop=(kp == 1),
                            )
                        if conv_eng is nc.scalar:
                            nc.scalar.activation(vT_f8[:, jt // 2, jt % 2, :], pv[:], AF.Identity)
                        else:
                            nc.vector.tensor_copy(vT_f8[:, jt // 2, jt % 2, :], pv[:])

                def u_proj(ich):
                    # u = (Wk^T Wq) hn + bu for this query chunk; chunk 0's
                    # converts run on ACT (DVE is stats-busy at the head)
                    for co in range(CT):
                        pq = ps.tile([P, 512], F32, tag="pp", bufs=2, name=f"pq_{ich}_{co}")
                        for kp in range(2):
                            nc.tensor.matmul(
                                pq[:], lhsT=wkqt_f8[:, kp, :, co * P:(co + 1) * P],
                                rhs=hn_f8[:, kp, :, ich * 512:(ich + 1) * 512],
                                perf_mode=mybir.MatmulPerfMode.DoubleRow,
                                start=(kp == 0), stop=(kp == 1),
                            )
                        if ich == 0 and co < 2:
                            nc.scalar.activation(
                                u_f8[:, co // 2, co % 2, ich * 512:(ich + 1) * 512], pq[:],
                                AF.Identity, bias=bu4[:, co:co + 1],
                            )
                        else:
                            nc.vector.tensor_scalar(
                                u_f8[:, co // 2, co % 2, ich * 512:(ich + 1) * 512], pq[:],
                                bu4[:, co:co + 1], None, ALU.add,
                            )

                at2s = {ich: [] for ich in range(ICH)}
                rbs = {}

                def att_slice(ich, t):
                    # scores pair -> one exp (den accumulated post-loop)
                    sc = ps.tile([P, 2, 512], F32, tag="sc", bufs=2, name=f"sc_{ich}_{t}")
                    for s in range(2):
                        jt = 2 * t + s
                        for kp in range(2):
                            nc.tensor.matmul(
                                sc[:, s, :], lhsT=hn_f8[:, kp, :, jt * P:(jt + 1) * P],
                                rhs=u_f8[:, kp, :, ich * 512:(ich + 1) * 512],
                                perf_mode=mybir.MatmulPerfMode.DoubleRow,
                                start=(kp == 0), stop=(kp == 1),
                            )
                    at2 = atp.tile([P, 2, 512], FP8, tag="at", name=f"at2_{ich}_{t}")
                    at2s[ich].append(at2)
                    nc.scalar.activation(at2[:], sc[:], AF.Exp, scale=SCALE, bias=bias_m1[:])

                def den_rec(ich, psum_tag="pp"):
                    den = ps.tile([P, 512], F32, tag=psum_tag, bufs=2, name=f"den_{ich}")
                    for t in range(TT):
                        nc.tensor.matmul(
                            den[:], lhsT=ones_f8[:], rhs=at2s[ich][t][:],
                            perf_mode=mybir.MatmulPerfMode.DoubleRow,
                            start=(t == 0), stop=(t == TT - 1),
                        )
                    rb = finp.tile([P, 512], F32, tag="rb", name=f"rb_{ich}")
                    nc.vector.reciprocal(rb[:], den[:])
                    rbs[ich] = rb

                def av_chain(ich, ct, psum_tag="o2"):
                    o2t = ps.tile([P, 512], F32, tag=psum_tag, bufs=2, name=f"o2_{ich}_{ct}")
                    for t in range(TT):
                        nc.tensor.matmul(
                            o2t[:], lhsT=vT_f8[:, t, :, ct * P:(ct + 1) * P],
                            rhs=at2s[ich][t][:],
                            perf_mode=mybir.MatmulPerfMode.DoubleRow,
                            start=(t == 0), stop=(t == TT - 1),
                        )
                    t1 = finp.tile([P, 512], F32, tag="t1", name=f"t1_{ich}_{ct}")
                    nc.vector.tensor_tensor(t1[:], o2t[:], rbs[ich][:], ALU.mult)
                    fin = finp.tile([P, 512], F32, tag="fin", name=f"fin_{ich}_{ct}")
                    nc.vector.scalar_tensor_tensor(
                        fin[:], t1[:], bpe4[:, ct:ct + 1],
                        x_sb[:, ct, ich * 512:(ich + 1) * 512],
                        ALU.add, ALU.add,
                    )
                    out_q = nc.sync if ct % 2 == 0 else nc.scalar
                    out_q.dma_start(
                        out_d[ct * P:(ct + 1) * P, ich * 512:(ich + 1) * 512], fin[:]
                    )

                # --- chunk 0 window: phase 1 interleaved with its slices ---
                hn_conv(0, split=True)
                u_proj(0)
                v_chains(0, nc.vector)
                for jc in range(1, JC):
                    hn_conv(jc)
                    if jc <= 5:
                        # one chain converted on ACT, three on DVE
                        v_chains(jc, nc.vector)
                    for t in (2 * (jc - 1), 2 * (jc - 1) + 1):
                        att_slice(0, t)
                for t in (2 * (JC - 1), 2 * (JC - 1) + 1):
                    att_slice(0, t)

                # --- chunks 1..3: slices with drained finish work ---
                pend = [lambda: v_chains(6, nc.vector),
                        lambda: v_chains(7, nc.scalar),
                        lambda: den_rec(0),
                        lambda: av_chain(0, 0), lambda: av_chain(0, 1),
                        lambda: u_proj(2),
                        lambda: av_chain(0, 2), lambda: av_chain(0, 3)]
                u_proj(1)
                last = ICH - 1
                tmaj = {}

                def tmaj_start(ct, upto):
                    o2t = ps.tile([P, 512], F32, tag="o2", bufs=2, name=f"o2_{last}_{ct}")
                    tmaj[ct] = o2t
                    for t in range(upto):
                        nc.tensor.matmul(
                            o2t[:], lhsT=vT_f8[:, t, :, ct * P:(ct + 1) * P],
                            rhs=at2s[last][t][:],
                            perf_mode=mybir.MatmulPerfMode.DoubleRow,
                            start=(t == 0), stop=False,
                        )

                for ich in range(1, ICH):
                    den_l = None
                    for t in range(TT):
                        att_slice(ich, t)
                        if ich == last:
                            if t == 4:
                                den_l = ps.tile([P, 512], F32, tag="pp", bufs=2,
                                                name=f"den_{last}")
                                for tp in range(5):
                                    nc.tensor.matmul(
                                        den_l[:], lhsT=ones_f8[:], rhs=at2s[last][tp][:],
                                        perf_mode=mybir.MatmulPerfMode.DoubleRow,
                                        start=(tp == 0), stop=False,
                                    )
                            elif t > 4:
                                nc.tensor.matmul(
                                    den_l[:], lhsT=ones_f8[:], rhs=at2s[last][t][:],
                                    perf_mode=mybir.MatmulPerfMode.DoubleRow,
                                    start=False, stop=(t == TT - 1),
                                )
                            if t == 12:
                                tmaj_start(0, 13)
                                tmaj_start(1, 13)
                            elif t > 12:
                                for ct in (0, 1):
                                    nc.tensor.matmul(
                                        tmaj[ct][:], lhsT=vT_f8[:, t, :, ct * P:(ct + 1) * P],
                                        rhs=at2s[last][t][:],
                                        perf_mode=mybir.MatmulPerfMode.DoubleRow,
                                        start=False, stop=(t == TT - 1),
                                    )
                        if t % 2 == 1 and pend:
                            pend.pop(0)()
                    while pend:
                        pend.pop(0)()
                    if ich < ICH - 1:
                        pend = [lambda i=ich: den_rec(i),
                                lambda i=ich: av_chain(i, 0),
                                lambda i=ich: av_chain(i, 1)]
                        if ich + 2 < ICH:
                            pend.append(lambda i=ich: u_proj(i + 2))
                        pend += [lambda i=ich: av_chain(i, 2),
                                 lambda i=ich: av_chain(i, 3)]
                # tail: reciprocal, finish t-major ct0/1, then ct2/3 on sc ring
                rb = finp.tile([P, 512], F32, tag="rb", name=f"rb_{last}")
                nc.vector.reciprocal(rb[:], den_l[:])
                rbs[last] = rb
                for ct in (0, 1):
                    t1 = finp.tile([P, 512], F32, tag="t1", name=f"t1_{last}_{ct}")
                    nc.vector.tensor_tensor(t1[:], tmaj[ct][:], rb[:], ALU.mult)
                    fin = finp.tile([P, 512], F32, tag="fin", name=f"fin_{last}_{ct}")
                    nc.vector.scalar_tensor_tensor(
                        fin[:], t1[:], bpe4[:, ct:ct + 1],
                        x_sb[:, ct, last * 512:(last + 1) * 512],
                        ALU.add, ALU.add,
                    )
                    out_q = nc.sync if ct % 2 == 0 else nc.scalar
                    out_q.dma_start(
                        out_d[ct * P:(ct + 1) * P, last * 512:(last + 1) * 512], fin[:]
                    )
                av_chain(last, 2, psum_tag="sc")
                av_chain(last, 3, psum_tag="sc")
    return nc


_NC = None


def _get_nc():
    global _NC
    if _NC is None:
        _NC = build_nc()
    return _NC


def _make_in_maps(x, gamma, beta, wq, bq, wk, bk, wv, bv, wp, bp):
    x = np.ascontiguousarray(np.asarray(x, dtype=np.float32)).reshape(4, C, N)
    bf = ml_dtypes.bfloat16

    def pack8(w):
        return np.ascontiguousarray(
            np.asarray(w, np.float32).T.reshape(2, 2, P, 512).transpose(2, 0, 1, 3)
            .reshape(P, 4 * 512).astype(mybir.dt.np(FP8))
        )

    g4i = np.zeros((P, 4), np.float32)
    for p in range(P):
        g4i[p, p // GSIZE] = 1.0
    g4 = g4i / GSIZE          # group-mean matmul (pre-scaled)
    g4t = np.ascontiguousarray(g4i.T)  # broadcast indicator (0/1)

    wq = np.asarray(wq, np.float32)
    wk = np.asarray(wk, np.float32)
    wv = np.asarray(wv, np.float32)
    wp = np.asarray(wp, np.float32)
    # scores fold: u = (Wk^T Wq) hn + Wk^T bq  (bk shift is softmax-invariant)
    wkq = wk.T @ wq
    bu = wk.T @ np.asarray(bq, np.float32)
    # output fold: attn@v with (Wp Wv) hn; bpe = Wp bv + bp
    wpv = wp @ wv
    bpe = np.asarray(bp, np.float32) + wp @ np.asarray(bv, np.float32)

    common = {
        "wkqtf8": pack8(wkq), "wpvtf8": pack8(wpv),
        "gamma": np.asarray(gamma, np.float32), "beta": np.asarray(beta, np.float32),
        "bu": np.ascontiguousarray(bu), "bpe": np.ascontiguousarray(bpe),
        "g4": g4, "g4t": g4t,
    }
    in_maps = []
    for core in range(8):
        bidx, half = core // 2, core % 2
        xb = x[bidx]
        if half == 0:
            xp = xb
        else:
            xp = np.concatenate([xb[:, NQ:], xb[:, :NQ]], axis=1)
        xp = np.ascontiguousarray(xp)
        in_maps.append({"xbf": xp.astype(bf), **common})
    return in_maps


def run(inputs, trace=False):
    nc = _get_nc()
    in_maps = _make_in_maps(**inputs)
    res = run_bass_kernel_spmd(nc, in_maps, list(range(8)), trace=trace)
    out = np.empty((4, C, N), np.float32)
    for core in range(8):
        bidx, half = core // 2, core % 2
        o = res.results[core]["out"]
        if half == 0:
            out[bidx, :, :NQ] = o
        else:
            out[bidx, :, NQ:] = o
    return out.reshape(4, C, 64, 64), res


def kernel(**inputs):
    out, _ = run(inputs, trace=False)
    return out
